# revision 20
# baseline (speedup 1.0000x reference)
"""Bass/Trainium2 kernel for nn_AdjModel (GNN message passing).

Data-parallel over batch: 8 graphs -> 8 NeuronCores, one graph per core.
Host only reshapes/pads value tensors and precomputes integer gather-index
layouts; all math runs on device.

Layouts per core (V=5000 padded to VP=5120, T=40 node tiles of 128):
  node-major [128, T, c] : partition p, tile t  <-> node v = t*128 + p
  mcf-major  [128, 40]   : partition 16g+r, slot s <-> node u = 640g+16s+r
"""
import sys

sys.path.insert(0, "/opt/trn_rl_repo")
sys.path.insert(0, "/root/problem")

import numpy as np

import concourse.bass as bass
import concourse.bacc as bacc
import concourse.bass_isa as bass_isa
import concourse.mybir as mybir
import concourse.tile as tile
from concourse.vector_clock import ScopedClock


# ---- inlined tilefix (walrus here allows only 1 sync-wait per Drain) ----
"""Patch TileContext._drain_and_barrier: the walrus in this container only
accepts ONE sync-wait command on a Drain (CoreV3 setupSyncWait), but Tile's
final drain attaches a wait per live semaphore. Distribute the extra waits
across a chain of sync-engine NOPs placed right after the drain (same
sequencer => executed in order before the all-engine barrier + sem clears).
"""


def _patched_drain_and_barrier(self, tick_clock, wait_clock):
    nc = self.nc
    drain_inst = nc.sync.drain()
    wait_clock.add_sem_waits(drain_inst.ins, ScopedClock({None: tick_clock.global_clock}))
    si = drain_inst.ins.sync_info
    waits = list(si.on_wait) if si is not None else []
    if len(waits) > 1:
        si.on_wait = [waits[0]]
        rest = waits[1:]
        # Find any semaphore handle to seed each nop's sync_info, then
        # overwrite the wait list with the real SyncWait entries.
        assert self.sems is not None
        any_sem = next(iter(self.sems.allocated().values()))
        i = 0
        while i < len(rest):
            nop = nc.sync.nop(nofuse=True, hint="drain_wait_spill")
            nop.wait_op(any_sem, 0, "sem-ge", check=False)
            take = rest[i : i + 1]
            nop.ins.sync_info.on_wait = take
            i += 1

    nc.all_engine_barrier()
    assert self.sems is not None
    popped = nc._tile_sem_poison_stack.pop()
    assert popped is self._sem_poison
    nc.clear_and_free_semaphores(list(self.sems.allocated().values()))
    nc.all_engine_barrier()


def install():
    tile.TileContext._drain_and_barrier = _patched_drain_and_barrier


def install_ntff_hook():
    """The agent image's `antenv` lacks `axon_hooks`, so trace=True degrades.
    Recreate the module and register the ctypes NTFF hook so neuron-profile
    exec_time_ns works under axon."""
    import sys, types

    if "antenv.axon_hooks" in sys.modules:
        return
    mod = types.ModuleType("antenv.axon_hooks")
    _hook = [None]

    def set_axon_ntff_profile_hook(h):
        _hook[0] = h

    def get_axon_ntff_profile_hook():
        return _hook[0]

    mod.set_axon_ntff_profile_hook = set_axon_ntff_profile_hook
    mod.get_axon_ntff_profile_hook = get_axon_ntff_profile_hook
    sys.modules["antenv.axon_hooks"] = mod
    try:
        from trn_agent_boot.trn_boot import _ntff_profile_via_ctypes

        set_axon_ntff_profile_hook(_ntff_profile_via_ctypes("/opt/axon/libaxon_pjrt.so"))
    except Exception as e:
        print("ntff hook install failed:", e)

install()
install_ntff_hook()
from concourse.bass_utils import run_bass_kernel_spmd

mdt = mybir.dt
AT = mybir.ActivationFunctionType
ALU = mybir.AluOpType
AX = mybir.AxisListType

B, V, D = 8, 5000, 10
EMB, F, E, H = 32, 2, 64, 4
DH = E // H
VP, T = 5120, 40
LAYERS, FLOW_ITERS, DUAL_ITERS = 2, 10, 10
STEP, MOM = 0.01, 0.9
BIG = 1e9
CH = 4            # node tiles per chunk
NCHUNK = T // CH  # 10
ISQ = 1.0 / 4.0   # 1/sqrt(dh)
LS_CHAINS = 5     # local_scatter chain passes (max same-source dup depth 6)


# ---------------- host-side layout helpers ----------------

def wrap16(lst):
    """dma_gather index layout: list[i] -> sbuf[i % 16, i // 16], replicated
    across the 8 gpsimd partition groups."""
    a = np.asarray(lst, np.int16)
    n = len(a)
    assert n % 16 == 0
    a = a.reshape(n // 16, 16).T  # [16, n/16]
    return np.ascontiguousarray(np.tile(a, (8, 1)))  # [128, n/16]


def groupwrap16(lists):
    """ap_gather per-core lists: lists[g] wrapped into partitions 16g..16g+15."""
    rows = []
    for g in range(8):
        a = np.asarray(lists[g], np.int16)
        rows.append(a.reshape(len(a) // 16, 16).T)
    return np.ascontiguousarray(np.concatenate(rows, axis=0))


def node_major(a2d):
    c = a2d.shape[1]
    out = np.zeros((VP, c), a2d.dtype)
    out[: a2d.shape[0]] = a2d
    return np.ascontiguousarray(
        out.reshape(T, 128, c).transpose(1, 0, 2).reshape(128, T * c)
    )


def mcf_major(a1d):
    out = np.zeros(VP, a1d.dtype)
    out[: a1d.shape[0]] = a1d
    return np.ascontiguousarray(out.reshape(128, 40))


def edge_list_chunks(src):
    """src [VP, D] int -> flat gather list with chunk-of-CH-tiles node-major order."""
    lst = np.empty(VP * D, np.int64)
    for c in range(NCHUNK):
        base = c * CH * 128 * D
        for tt in range(CH):
            t = c * CH + tt
            for nb in range(D):
                j = tt * D + nb
                lst[base + j * 128 : base + (j + 1) * 128] = src[t * 128 : (t + 1) * 128, nb]
    return lst


def _shared_prep(inputs):
    import ml_dtypes

    f32 = np.float32

    def b16(x):
        return np.ascontiguousarray(np.asarray(x, f32).astype(ml_dtypes.bfloat16))

    emb = np.asarray(inputs["node_embedding_var"], f32)
    s = {}
    embT = np.zeros((EMB, VP), f32)
    embT[:, :V] = emb.T
    s["embT"] = embT
    s["embN"] = node_major(emb)
    W_enc = np.asarray(inputs["W_enc"], f32)
    s["Wenc1"] = np.ascontiguousarray(W_enc[:EMB])
    s["Wenc2"] = np.ascontiguousarray(W_enc[EMB:])
    s["benc"] = np.asarray(inputs["b_enc"], f32)[None]
    s["Wqkv"] = b16(np.concatenate([inputs["Wq"], inputs["Wk"], inputs["Wv"]], 1))
    s["Wo16"] = b16(inputs["Wo"])
    s["Wgru"] = b16(inputs["W_gru"])
    s["Ugru"] = b16(inputs["U_gru"])
    s["bgru"] = np.asarray(inputs["b_gru"], f32)[None]
    Wd1 = np.asarray(inputs["Wd1"], f32)
    s["Wdec"] = b16(np.concatenate([Wd1[:E], Wd1[E:], np.asarray(inputs["Wu1"], f32)], 1))
    s["bdec"] = np.concatenate(
        [np.zeros(32, f32), np.zeros(32, f32), np.asarray(inputs["bu1"], f32)]
    )[None]  # u1: no bias; y: bd1 folded into yd rows; udual: bu1
    s["Wd2rep"] = np.ascontiguousarray(np.tile(np.asarray(inputs["Wd2"], f32).T, (128, 1)))
    s["Wu2rep"] = np.ascontiguousarray(np.tile(np.asarray(inputs["Wu2"], f32).T, (128, 1)))
    s["bd2"] = float(np.asarray(inputs["bd2"]).reshape(-1)[0])
    s["bu2"] = float(np.asarray(inputs["bu2"]).reshape(-1)[0])
    bd1v = np.asarray(inputs["bd1"], f32)
    pad = np.zeros((120, 64), f32)
    pad[:, 0:32] = bd1v[None, :]
    s["ydpad120"] = pad
    s["identb"] = b16(np.eye(128, dtype=f32))
    s["ones1"] = np.ones((1, 128), f32)
    return s


def _host_prep(core, shared):
    f32 = np.float32
    import ml_dtypes

    demands = np.asarray(core["demands"], f32)[:, 0]
    feat = np.asarray(core["node_features"], f32)
    adj = np.asarray(core["adj_lst"], np.int64)
    in_idx = np.asarray(core["in_indices"], np.int64)
    rev_idx = np.asarray(core["rev_indices"], np.int64)
    num_nodes = int(core["num_nodes"])

    m = dict(shared)
    m["demN"] = node_major(demands[:, None])
    m["demM"] = mcf_major(demands)
    featT = np.zeros((F, VP), f32)
    featT[:, :V] = feat.T
    m["featT"] = featT

    maskf = (adj == num_nodes).astype(f32)
    m["maskN"] = node_major(maskf)
    m["nbigN"] = node_major(maskf * BIG)
    pm = np.zeros((VP, 1), f32)
    pm[:V] = 1.0
    m["pmaskT"] = node_major(pm)
    m["pmaskN"] = node_major(np.repeat(pm, D, axis=1))

    adjc = np.full((VP, D), V, np.int64)
    adjc[:V] = adj
    m["adjIdx"] = wrap16(edge_list_chunks(adjc))

    iv = np.zeros((VP, D), np.int64)
    iv[:V] = in_idx[..., 1]
    ip = np.zeros((VP, D), np.int64)
    ip[:V] = in_idx[..., 2]
    riv = np.zeros((VP, D), np.int64)
    riv[:V] = rev_idx[..., 1]
    rip = np.zeros((VP, D), np.int64)
    rip[:V] = rev_idx[..., 2]

    # local_scatter indices for the s-iteration: partition p owns dest nodes
    # [40p, 40p+40); edge (v, d) lives at slot (v%40)*10 + d. Pass 1 scatters
    # table position u -> first slot wanting s[u]; chain pass k copies slot of
    # use k to slot of use k+1 within the same (p, u) cell.
    v_arr = np.repeat(np.arange(V), D)
    d_arr = np.tile(np.arange(D), V)
    u_arr = iv[:V].reshape(-1)
    p_arr = v_arr // 40
    slot_arr = (v_arr % 40) * 10 + d_arr
    key = p_arr * VP + u_arr
    order = np.argsort(key, kind="stable")
    ks, sl = key[order], slot_arr[order]
    new_grp = np.r_[True, ks[1:] != ks[:-1]]
    grp_start = np.flatnonzero(new_grp)
    grp_id = np.cumsum(new_grp) - 1
    rank = np.arange(len(ks)) - grp_start[grp_id]
    assert rank.max() <= LS_CHAINS, f"dup chain depth {rank.max()} > {LS_CHAINS}"
    ls1 = np.full((128, VP), -1, np.int16)
    lsC = np.full((LS_CHAINS, 128, 400), -1, np.int16)
    m0 = rank == 0
    ls1[ks[m0] // VP, ks[m0] % VP] = sl[m0]
    for k in range(1, int(rank.max()) + 1):
        mk = rank == k
        pos = np.flatnonzero(mk)
        lsC[k - 1, ks[mk] // VP, sl[pos - 1]] = sl[mk]
    m["lsIdx1"] = ls1
    m["lsIdxC"] = np.ascontiguousarray(lsC.transpose(1, 0, 2).reshape(128, LS_CHAINS * 400))

    # pair-gather indices for the edge-weight (wM, mcf-major) and reverse-flow
    # (rvf, node-major) gathers. Flat fp16 table order e = (v%128)*400 +
    # (v//128)*10 + d (contiguous per node-major partition); ap_gather fetches
    # fp16 pairs at e>>1, parity masks select the half.
    dd = np.arange(D)[None, None, :]
    # mcf-major: partition P owns nodes 40P+s
    vv = 40 * np.arange(128)[:, None, None] + np.arange(40)[None, :, None]  # [128,40,1]
    uw = iv[vv, dd]  # [128, 40, 10]
    ew = (uw % 128) * 400 + (uw // 128) * 10 + ip[vv, dd]
    wpair = (ew >> 1).astype(np.int16)
    m["wpIdx"] = groupwrap16(
        [wpair[16 * g : 16 * g + 16].transpose(1, 0, 2).reshape(6400) for g in range(8)]
    )
    wparO = (ew & 1).astype(f32).reshape(128, 400)
    m["wParO"] = wparO
    m["wParE"] = 1.0 - wparO
    # node-major: partition p owns nodes t*128+p
    vvn = np.arange(128)[:, None, None] + 128 * np.arange(40)[None, :, None]
    ur = riv[vvn, dd]
    er = (ur % 128) * 400 + (ur // 128) * 10 + rip[vvn, dd]
    rpair = (er >> 1).astype(np.int16)
    m["rpIdx"] = groupwrap16(
        [rpair[16 * g : 16 * g + 16].transpose(1, 0, 2).reshape(6400) for g in range(8)]
    )
    rparO = (er & 1).astype(f32).reshape(128, 400)
    m["rParO"] = rparO
    m["rParE"] = 1.0 - rparO
    return m


# ---------------- device program ----------------

def build(nc, bd2, bu2, phase=5):
    f32, bf = mdt.float32, mdt.bfloat16

    def din(name, shape, dt=f32):
        return nc.dram_tensor(name, list(shape), dt, kind="ExternalInput").ap()

    embT = din("embT", [EMB, VP])
    embN = din("embN", [128, T * EMB])
    featT = din("featT", [F, VP])
    demN = din("demN", [128, T])
    demM = din("demM", [128, T])
    maskN = din("maskN", [128, T * D])
    nbigN = din("nbigN", [128, T * D])
    pmaskT = din("pmaskT", [128, T])
    pmaskN = din("pmaskN", [128, T * D])
    adjIdx = din("adjIdx", [128, 3200], mdt.int16)
    lsIdx1 = din("lsIdx1", [128, VP], mdt.int16)
    lsIdxC = din("lsIdxC", [128, LS_CHAINS * 400], mdt.int16)
    wpIdx = din("wpIdx", [128, 400], mdt.int16)
    rpIdx = din("rpIdx", [128, 400], mdt.int16)
    wParE = din("wParE", [128, 400])
    wParO = din("wParO", [128, 400])
    rParE = din("rParE", [128, 400])
    rParO = din("rParO", [128, 400])
    Wenc1 = din("Wenc1", [EMB, E])
    Wenc2 = din("Wenc2", [F, E])
    benc = din("benc", [1, E])
    Wqkv = din("Wqkv", [E, 3 * E], bf)
    Wo16 = din("Wo16", [E, E], bf)
    Wgru = din("Wgru", [E, 3 * E], bf)
    Ugru = din("Ugru", [E, 3 * E], bf)
    bgru = din("bgru", [1, 3 * E])
    Wdec = din("Wdec", [E, 96], bf)
    bdec = din("bdec", [1, 96])
    Wd2rep = din("Wd2rep", [128, 32])
    Wu2rep = din("Wu2rep", [128, 32])
    ydpad120 = din("ydpad120", [120, 64])
    identB = din("identb", [128, 128], bf)
    ones1 = din("ones1", [1, 128])

    out = nc.dram_tensor("out", [1, 1], f32, kind="ExternalOutput").ap()
    dbg = nc.dram_tensor("dbg", [128, 2560], f32, kind="ExternalOutput").ap()

    with tile.TileContext(nc) as tc:
        import contextlib

        ctx = contextlib.ExitStack()
        sb = ctx.enter_context(tc.tile_pool(name="sb", bufs=1))
        dram = ctx.enter_context(tc.tile_pool(name="dram", bufs=1, space="DRAM"))
        midctx = contextlib.ExitStack()
        mid = midctx.enter_context(tc.tile_pool(name="mid", bufs=1))

        S, A, Vc, P, PE = nc.sync, nc.scalar, nc.vector, nc.gpsimd, nc.tensor

        # persistent state
        x = mid.tile([128, T * E], f32, name="x")
        u1 = mid.tile([128, T * 32], f32, name="u1")
        xT = mid.tile([E, VP], bf, name="xT")
        q = mid.tile([128, T * E], bf, name="q")
        maskf = sb.tile([128, T * D], f32, name="maskf")
        nbig = sb.tile([128, T * D], f32, name="nbig")
        pmT = sb.tile([128, T], f32, name="pmT")
        pmN = sb.tile([128, T * D], f32, name="pmN")
        zkv = mid.tile([128, 128], bf, name="zkv")
        onesb = sb.tile([1, 128], f32, name="onesb")
        identb = mid.tile([128, 128], bf, name="identb_sb")
        adjI = mid.tile([128, 3200], mdt.int16, name="adjI")
        nW = sb.tile([128, T * D], f32, name="nW")
        dvN = sb.tile([128, T], f32, name="dvN")
        ddf = sb.tile([128, T * D], f32, name="ddf")
        dp = sb.tile([128, T], f32, name="dp")
        dpM = sb.tile([128, T], f32, name="dpM")
        demNs = sb.tile([128, T], f32, name="demNs")
        wq16 = sb.tile([E, 3 * E], bf, name="wq16")
        wo16s = sb.tile([E, E], bf, name="wo16s")
        wg16 = sb.tile([E, 3 * E], bf, name="wg16")
        ug16 = sb.tile([E, 3 * E], bf, name="ug16")
        wd16 = sb.tile([E, 96], bf, name="wd16")
        bg = sb.tile([1, 3 * E], f32, name="bg")
        bdc = sb.tile([1, 96], f32, name="bdc")
        wd2r = sb.tile([128, 32], f32, name="wd2r")
        wu2r = sb.tile([128, 32], f32, name="wu2r")

        kvDs = [dram.tile([VP, 128], bf, name=f"kvD{l}") for l in range(LAYERS)]
        ydD = dram.tile([VP, E], f32, name="ydD")
        sfD = dram.tile([VP, 1], f32, name="sfD")
        sfD16 = dram.tile([VP, 1], mdt.float16, name="sfD16")
        wpD = dram.tile([VP * D, 1], mdt.float16, name="wpD")

        nidxreg = P.alloc_register("nidxreg")
        P.reg_mov(nidxreg, CH * D * 128)
        S.dma_start(onesb[:], ones1)
        S.dma_start(identb[:], identB)
        S.dma_start(adjI[:], adjIdx)
        S.dma_start(maskf[:], maskN)
        S.dma_start(nbig[:], nbigN)
        S.dma_start(pmT[:], pmaskT)
        S.dma_start(pmN[:], pmaskN)
        P.memset(zkv[:], 0.0)
        S.dma_start(wq16[:], Wqkv)
        S.dma_start(wo16s[:], Wo16)
        S.dma_start(wg16[:], Wgru)
        S.dma_start(ug16[:], Ugru)
        S.dma_start(bg[:], bgru)
        S.dma_start(wd16[:], Wdec)
        S.dma_start(bdc[:], bdec)
        S.dma_start(wd2r[:], Wd2rep)
        S.dma_start(wu2r[:], Wu2rep)
        S.dma_start(demNs[:], demN)

        # ---------------- encode ----------------
        with tc.tile_pool(name="encp", bufs=2) as enc, tc.tile_pool(
            name="encps", bufs=2, space="PSUM"
        ) as eps:
            embTs = enc.tile([EMB, VP], f32, name="embTs")
            featTs = enc.tile([F, VP], f32, name="featTs")
            embNs = enc.tile([128, T * EMB], f32, name="embNs")
            w1 = enc.tile([EMB, E], f32, name="w1")
            w2 = enc.tile([F, E], f32, name="w2")
            be = enc.tile([1, E], f32, name="be")
            demMs = enc.tile([128, T], f32, name="demMs")
            S.dma_start(embTs[:], embT)
            S.dma_start(featTs[:], featT)
            S.dma_start(embNs[:], embN)
            S.dma_start(w1[:], Wenc1)
            S.dma_start(w2[:], Wenc2)
            S.dma_start(be[:], benc)
            S.dma_start(demMs[:], demM)

            A.activation(dp[:], demNs[:], AT.Relu)
            A.activation(dpM[:], demMs[:], AT.Relu)

            sqv = enc.tile([128, T * EMB], f32, name="sqv")
            Vc.tensor_mul(sqv[:], embNs[:], embNs[:])
            n2 = enc.tile([128, T], f32, name="n2")
            Vc.reduce_sum(n2[:], sqv[:].rearrange("p (t c) -> p t c", c=EMB), axis=AX.X)
            nrm = enc.tile([128, T], f32, name="nrm")
            A.activation(nrm[:], n2[:], AT.Sqrt)
            Vc.tensor_scalar_max(out=nrm[:], in0=nrm[:], scalar1=1.0)
            scl = enc.tile([128, T], f32, name="scl")
            Vc.reciprocal(scl[:], nrm[:])

            for t in range(T):
                p1 = eps.tile([128, E], f32, name="p1")
                p2 = eps.tile([128, E], f32, name="p2")
                PE.matmul(p1[:], embTs[:, t * 128 : (t + 1) * 128], w1[:], start=True, stop=True)
                PE.matmul(p2[:], featTs[:, t * 128 : (t + 1) * 128], w2[:], start=True, stop=False)
                PE.matmul(p2[:], onesb[:], be[:], start=False, stop=True)
                A.activation(x[:, t * E : (t + 1) * E], p2[:], AT.Copy)
                Vc.scalar_tensor_tensor(
                    out=x[:, t * E : (t + 1) * E], in0=p1[:], scalar=scl[:, t : t + 1],
                    in1=x[:, t * E : (t + 1) * E], op0=ALU.mult, op1=ALU.add,
                )
                xb = enc.tile([128, E], bf, name="xb")
                A.activation(xb[:], x[:, t * E : (t + 1) * E], AT.Copy)
                xtp = eps.tile([E, 128], bf, name="xtp")
                PE.transpose(xtp[:], xb[:], identb[:])
                A.activation(xT[:, t * 128 : (t + 1) * 128], xtp[:], AT.Copy)

        if phase <= 1:
            S.dma_start(dbg[:], x[:])
            S.dma_start(out, x[0:1, 0:1])
            midctx.close()
            ctx.close()
            return nc
        # ---------------- graph layers ----------------
        layctx = contextlib.ExitStack()
        kvps = layctx.enter_context(tc.tile_pool(name="kvps", bufs=2, space="PSUM"))
        for layer in range(LAYERS):
            with tc.tile_pool(name=f"lay{layer}", bufs=4) as lp, tc.tile_pool(
                name=f"lps{layer}", bufs=1, space="PSUM"
            ) as lps:
                kvD = kvDs[layer]

                def kv_chunk(c, dstD):
                    kvc = lp.tile([128, CH, 128], bf, name="kvc")
                    for tt in range(CH):
                        t = c * CH + tt
                        pq = kvps.tile([128, 3 * E], f32, name="pq")
                        PE.matmul(pq[:], xT[:, t * 128 : (t + 1) * 128], wq16[:], start=True, stop=True)
                        A.activation(q[:, t * E : (t + 1) * E], pq[:, :E], AT.Copy)
                        Vc.tensor_copy(kvc[:, tt, :], pq[:, E:])
                    S.dma_start(
                        dstD[:].rearrange("(t p) c -> p t c", p=128)[:, c * CH : (c + 1) * CH, :],
                        kvc[:],
                    )

                def dec_chunk(c):
                    ydc = lp.tile([128, CH, 33], f32, name="ydc")
                    for tt in range(CH):
                        t = c * CH + tt
                        pd = lps.tile([128, 96], f32, name="pd")
                        PE.matmul(pd[:], xT[:, t * 128 : (t + 1) * 128], wd16[:], start=True, stop=False)
                        PE.matmul(pd[:], onesb[:], bdc[:], start=False, stop=True)
                        Vc.tensor_copy(u1[:, t * 32 : (t + 1) * 32], pd[:, 0:32])
                        Vc.tensor_copy(ydc[:, tt, 0:32], pd[:, 32:64])
                        th = lp.tile([128, 32], f32, name="th")
                        A.activation(th[:], pd[:, 64:96], AT.Tanh)
                        junk = lp.tile([128, 32], f32, name="junk")
                        Vc.tensor_mul(junk[:], th[:], wu2r[:])
                        Vc.reduce_sum(dvN[:, t : t + 1], junk[:], axis=AX.X)
                        Vc.tensor_scalar_add(out=dvN[:, t : t + 1], in0=dvN[:, t : t + 1], scalar1=bu2)
                        Vc.tensor_copy(ydc[:, tt, 32:33], dvN[:, t : t + 1])
                    S.dma_start(
                        ydD[:].rearrange("(t p) c -> p t c", p=128)[:, c * CH : (c + 1) * CH, 0:33],
                        ydc[:],
                    )

                if layer == 0:
                    for c in range(NCHUNK):
                        kv_chunk(c, kvD)
                    S.dma_start(
                        kvD[:].rearrange("(t p) c -> p t c", p=128)[8:128, T - 1, :],
                        zkv[8:128, :],
                    )

                for c in range(NCHUNK):
                    kvn = lp.tile([128, CH * D, 128], bf, name="kvn")
                    P.dma_gather(
                        kvn[:], kvD[:], adjI[:, c * 320 : (c + 1) * 320],
                        num_idxs=CH * D * 128, num_idxs_reg=nidxreg, elem_size=128, single_packet=False,
                    )
                    # scores
                    prodk = lp.tile([128, CH * D * E], bf, name="prodk")
                    qv = (
                        q[:, c * CH * E : (c + 1) * CH * E]
                        .rearrange("p (tt e) -> p tt e", e=E)
                        .unsqueeze(2)
                        .broadcast_to((128, CH, D, E))
                    )
                    Vc.tensor_mul(
                        prodk[:].rearrange("p (tt nb e) -> p tt nb e", nb=D, e=E),
                        kvn[:, :, 0:E].rearrange("p (tt nb) e -> p tt nb e", nb=D),
                        qv,
                    )
                    sc = lp.tile([128, CH * D * H], f32, name="sc")
                    Vc.reduce_sum(
                        sc[:].rearrange("p (tn h) -> p tn h", h=H),
                        prodk[:].rearrange("p (tnh dh) -> p tnh dh", dh=DH),
                        axis=AX.X,
                    )
                    scm = lp.tile([128, CH * D * H], f32, name="scm")
                    Vc.scalar_tensor_tensor(
                        out=scm[:].rearrange("p (tn h) -> p tn h", h=H),
                        in0=sc[:].rearrange("p (tn h) -> p tn h", h=H),
                        scalar=ISQ,
                        in1=nbig[:, c * CH * D : (c + 1) * CH * D].unsqueeze(2).broadcast_to((128, CH * D, H)),
                        op0=ALU.mult, op1=ALU.subtract,
                    )
                    ex = lp.tile([128, CH * D * H], f32, name="ex")
                    A.activation(ex[:], scm[:], AT.Exp)
                    zs = lp.tile([128, CH * H], f32, name="zs")
                    Vc.reduce_sum(
                        zs[:].rearrange("p (tt h) -> p tt h", h=H),
                        ex[:].rearrange("p (tt nb h) -> p tt h nb", nb=D, h=H),
                        axis=AX.X,
                    )
                    rz = lp.tile([128, CH * H], f32, name="rz")
                    Vc.reciprocal(rz[:], zs[:])
                    at = lp.tile([128, CH * D * H], f32, name="at")
                    Vc.tensor_mul(
                        at[:].rearrange("p (tt nb h) -> p tt nb h", nb=D, h=H),
                        ex[:].rearrange("p (tt nb h) -> p tt nb h", nb=D, h=H),
                        rz[:].rearrange("p (tt h) -> p tt h", h=H).unsqueeze(2).broadcast_to((128, CH, D, H)),
                    )
                    prodv = lp.tile([128, CH * D * E], f32, name="prodv")
                    Vc.tensor_mul(
                        prodv[:].rearrange("p (tt nb h dh) -> p tt nb h dh", nb=D, h=H, dh=DH),
                        kvn[:, :, E:].rearrange("p (tt nb) (h dh) -> p tt nb h dh", nb=D, h=H),
                        at[:].rearrange("p (tt nb h) -> p tt nb h", nb=D, h=H).unsqueeze(4).broadcast_to(
                            (128, CH, D, H, DH)
                        ),
                    )
                    agg = lp.tile([128, CH * E], f32, name="agg")
                    Vc.reduce_sum(
                        agg[:].rearrange("p (tt e) -> p tt e", e=E),
                        prodv[:].rearrange("p (tt nb e) -> p tt e nb", nb=D, e=E),
                        axis=AX.X,
                    )
                    # GRU per tile
                    for tt in range(CH):
                        t = c * CH + tt
                        aggb = lp.tile([128, E], bf, name="aggb")
                        A.activation(aggb[:], agg[:, tt * E : (tt + 1) * E], AT.Copy)
                        agT = lps.tile([E, 128], bf, name="trT")
                        PE.transpose(agT[:], aggb[:], identb[:])
                        agTs = lp.tile([E, 128], bf, name="agTs")
                        A.activation(agTs[:], agT[:], AT.Copy)
                        pnx = lps.tile([128, E], f32, name="pnx")
                        PE.matmul(pnx[:], agTs[:], wo16s[:], start=True, stop=True)
                        nxt = lp.tile([128, E], bf, name="nxt")
                        A.activation(nxt[:], pnx[:], AT.Tanh)
                        nxT = lps.tile([E, 128], bf, name="trT")
                        PE.transpose(nxT[:], nxt[:], identb[:])
                        nxTs = lp.tile([E, 128], bf, name="nxTs")
                        A.activation(nxTs[:], nxT[:], AT.Copy)
                        pA = lps.tile([128, 2 * E], f32, name="pA")
                        PE.matmul(pA[:], nxTs[:], wg16[:, : 2 * E], start=True, stop=False)
                        PE.matmul(pA[:], xT[:, t * 128 : (t + 1) * 128], ug16[:, : 2 * E], start=False, stop=False)
                        PE.matmul(pA[:], onesb[:], bg[:, : 2 * E], start=False, stop=True)
                        pBC = lps.tile([128, 2 * E], f32, name="pBC")
                        PE.matmul(pBC[:, :E], nxTs[:], wg16[:, 2 * E :], start=True, stop=False)
                        PE.matmul(pBC[:, :E], onesb[:], bg[:, 2 * E :], start=False, stop=True)
                        PE.matmul(pBC[:, E:], xT[:, t * 128 : (t + 1) * 128], ug16[:, 2 * E :], start=True, stop=True)
                        zr = lp.tile([128, 2 * E], f32, name="zr")
                        A.activation(zr[:], pA[:], AT.Sigmoid)
                        tmp = lp.tile([128, E], f32, name="tmp")
                        Vc.tensor_mul(tmp[:], zr[:, E:], pBC[:, E:])
                        Vc.tensor_add(out=tmp[:], in0=tmp[:], in1=pBC[:, :E])
                        hh = lp.tile([128, E], f32, name="hh")
                        A.activation(hh[:], tmp[:], AT.Tanh)
                        hmx = lp.tile([128, E], f32, name="hmx")
                        Vc.tensor_sub(out=hmx[:], in0=hh[:], in1=x[:, t * E : (t + 1) * E])
                        Vc.tensor_mul(hmx[:], hmx[:], zr[:, :E])
                        Vc.tensor_add(
                            out=x[:, t * E : (t + 1) * E], in0=x[:, t * E : (t + 1) * E], in1=hmx[:]
                        )
                        xb2 = lp.tile([128, E], bf, name="xb2")
                        A.activation(xb2[:], x[:, t * E : (t + 1) * E], AT.Copy)
                        xtp2 = lps.tile([E, 128], bf, name="xtp2")
                        PE.transpose(xtp2[:], xb2[:], identb[:])
                        A.activation(xT[:, t * 128 : (t + 1) * 128], xtp2[:], AT.Copy)
                    if layer + 1 < LAYERS:
                        kv_chunk(c, kvDs[layer + 1])
                    else:
                        dec_chunk(c)
                if layer + 1 < LAYERS:
                    S.dma_start(
                        kvDs[layer + 1][:].rearrange("(t p) c -> p t c", p=128)[8:128, T - 1, :],
                        zkv[8:128, :],
                    )

        layctx.close()
        if phase <= 2:
            S.dma_start(dbg[:], x[:])
            S.dma_start(out, x[0:1, 0:1])
            midctx.close()
            ctx.close()
            return nc
        # ---------------- decoders ----------------
        with tc.tile_pool(name="decp", bufs=3) as dpool, tc.tile_pool(
            name="decps", bufs=2, space="PSUM"
        ) as dps:
            # pad rows 5000..5119 = [bd1 | 0]
            ydp = dpool.tile([120, 64], f32, name="ydp")
            S.dma_start(ydp[:], ydpad120)
            S.dma_start(ydD[:].rearrange("(t p) c -> p t c", p=128)[8:128, T - 1, :], ydp[:])

            for c in range(NCHUNK):
                ydg = dpool.tile([128, CH * D, E], f32, name="ydg")
                P.dma_gather(
                    ydg[:], ydD[:], adjI[:, c * 320 : (c + 1) * 320],
                    num_idxs=CH * D * 128, num_idxs_reg=nidxreg, elem_size=E, single_packet=False,
                )
                h1p = dpool.tile([128, CH * D * 32], f32, name="h1p")
                u1v = (
                    u1[:, c * CH * 32 : (c + 1) * CH * 32]
                    .rearrange("p (tt k) -> p tt k", k=32)
                    .unsqueeze(2)
                    .broadcast_to((128, CH, D, 32))
                )
                h1p4 = h1p[:].rearrange("p (tt nb k) -> p tt nb k", nb=D, k=32)
                mk4 = (
                    maskf[:, c * CH * D : (c + 1) * CH * D]
                    .rearrange("p (tt nb) -> p tt nb", nb=D)
                    .unsqueeze(3)
                    .broadcast_to((128, CH, D, 32))
                )
                # h1p = u1*maskf ; then u1 - u1*maskf ; then + ydg
                Vc.tensor_mul(h1p4, u1v, mk4)
                Vc.tensor_sub(out=h1p4, in0=u1v, in1=h1p4)
                Vc.tensor_add(
                    out=h1p4,
                    in0=h1p4,
                    in1=ydg[:, :, 0:32].rearrange("p (tt nb) k -> p tt nb k", nb=D),
                )
                h1t = dpool.tile([128, CH * D * 32], f32, name="h1t")
                A.activation(h1t[:], h1p[:], AT.Tanh)
                pw = dpool.tile([128, CH * D * 32], f32, name="pw")
                Vc.tensor_mul(
                    pw[:].rearrange("p (tn k) -> p tn k", k=32),
                    h1t[:].rearrange("p (tn k) -> p tn k", k=32),
                    wd2r[:].unsqueeze(1).broadcast_to((128, CH * D, 32)),
                )
                nwc = dpool.tile([128, CH * D], f32, name="nwc")
                Vc.reduce_sum(nwc[:], pw[:].rearrange("p (tn k) -> p tn k", k=32), axis=AX.X)
                prd = dpool.tile([128, CH * D], f32, name="prd")
                Vc.tensor_sub(out=prd[:], in0=nwc[:], in1=nbig[:, c * CH * D : (c + 1) * CH * D])
                exn = dpool.tile([128, CH * D], f32, name="exn")
                A.activation(exn[:], prd[:], AT.Exp)
                zn = dpool.tile([128, CH], f32, name="zn")
                Vc.reduce_sum(zn[:], exn[:].rearrange("p (tt nb) -> p tt nb", nb=D), axis=AX.X)
                Vc.tensor_scalar_add(out=zn[:], in0=zn[:], scalar1=1e-30)
                rzn = dpool.tile([128, CH], f32, name="rzn")
                Vc.reciprocal(rzn[:], zn[:])
                Vc.tensor_mul(
                    nW[:, c * CH * D : (c + 1) * CH * D].rearrange("p (tt nb) -> p tt nb", nb=D),
                    exn[:].rearrange("p (tt nb) -> p tt nb", nb=D),
                    rzn[:].unsqueeze(2).broadcast_to((128, CH, D)),
                )
                # dual_diff = ydg[:, :, 32] - maskf*dv
                Vc.tensor_mul(
                    ddf[:, c * CH * D : (c + 1) * CH * D].rearrange("p (tt nb) -> p tt nb", nb=D),
                    maskf[:, c * CH * D : (c + 1) * CH * D].rearrange("p (tt nb) -> p tt nb", nb=D),
                    dvN[:, c * CH : (c + 1) * CH].unsqueeze(2).broadcast_to((128, CH, D)),
                )
                Vc.tensor_sub(
                    out=ddf[:, c * CH * D : (c + 1) * CH * D],
                    in0=ydg[:, :, 32],
                    in1=ddf[:, c * CH * D : (c + 1) * CH * D],
                )

        if phase <= 3:
            S.dma_start(dbg[:, 0:400], nW[:])
            S.dma_start(dbg[:, 400:800], ddf[:])
            S.dma_start(out, nW[0:1, 0:1])
            midctx.close()
            ctx.close()
            return nc
        # ---------------- mcf flow ----------------
        midctx.close()
        with tc.tile_pool(name="mcfp", bufs=1) as mp, tc.tile_pool(
            name="mcfps", bufs=2, space="PSUM"
        ) as mps:
            fp16 = mdt.float16
            from concourse.bass import _add_dep_helper

            lsI1s = mp.tile([128, VP], mdt.int16, name="lsI1s")
            S.dma_start(lsI1s[:], lsIdx1)
            lsICs = mp.tile([128, LS_CHAINS * 400], mdt.int16, name="lsICs")
            S.dma_start(lsICs[:], lsIdxC)
            wpIdxS = mp.tile([128, 400], mdt.int16, name="wpIdxS")
            S.dma_start(wpIdxS[:], wpIdx)
            rpIdxS = mp.tile([128, 400], mdt.int16, name="rpIdxS")
            S.dma_start(rpIdxS[:], rpIdx)
            wmE = mp.tile([128, 400], f32, name="wmE")
            S.dma_start(wmE[:], wParE)
            wmO = mp.tile([128, 400], f32, name="wmO")
            S.dma_start(wmO[:], wParO)
            rmE = mp.tile([128, 400], f32, name="rmE")
            S.dma_start(rmE[:], rParE)
            rmO = mp.tile([128, 400], f32, name="rmO")
            S.dma_start(rmO[:], rParO)

            wTab = mp.tile([128, VP * D], fp16, name="wTab")
            ev16 = mp.tile([128, 400], fp16, name="ev16")
            gpair = mp.tile([128, 12800], fp16, name="gpair")
            psel = mp.tile([128, 800], fp16, name="psel")

            def pair_gather(vals_f32, idx_tile, mE, mO, outA, outB):
                """flat fp16 table[e]: outA = even-half*mE, outB = odd-half*mO;
                caller sums. vals_f32 [128, 400] node-major edge values."""
                A.activation(ev16[:], vals_f32, AT.Copy)
                S.dma_start(wpD[:].rearrange("(p c) o -> p (c o)", p=128), ev16[:])
                S.dma_start(
                    wTab[:],
                    wpD[:].rearrange("(a v) o -> a (v o)", a=1).broadcast_to((128, VP * D)),
                )
                apg = P.ap_gather(
                    gpair[:], wTab[:], idx_tile[:],
                    channels=128, num_elems=VP * D // 2, d=2, num_idxs=6400,
                )
                for r in range(16):
                    eng = S if r % 2 == 0 else A
                    dmi = eng.dma_start(
                        psel[r:128:16, :].rearrange("p (s d c) -> p s d c", d=D, c=2),
                        gpair[r:128:16, :].rearrange(
                            "p (s r2 d c) -> p s r2 d c", r2=16, d=D, c=2
                        )[:, :, r, :, :],
                    )
                    _add_dep_helper(dmi.ins, apg.ins, sync=True, reason="extract")
                pv = psel[:].rearrange("p (s c) -> p s c", c=2)
                Vc.tensor_mul(outA, pv[:, :, 0], mE[:])
                Vc.tensor_mul(outB, pv[:, :, 1], mO[:])

            wM = mp.tile([128, 400], f32, name="wM")
            wMo = mp.tile([128, 400], f32, name="wMo")
            pair_gather(nW[:], wpIdxS, wmE, wmO, wM[:], wMo[:])
            Vc.tensor_add(out=wM[:], in0=wM[:], in1=wMo[:])
            fp16 = mdt.float16
            sTab16 = mp.tile([128, VP], fp16, name="sTab16")
            sM16 = mp.tile([128, T], fp16, name="sM16")
            sM = mp.tile([128, T], f32, name="sM")

            def rebuild_table16(src_mcf):
                A.activation(sM16[:], src_mcf, AT.Copy)
                S.dma_start(sfD16[:].rearrange("(p s) o -> p (s o)", p=128), sM16[:])
                S.dma_start(
                    sTab16[:],
                    sfD16[:].rearrange("(a v) o -> a (v o)", a=1).broadcast_to((128, VP)),
                )

            Vc.tensor_copy(sM[:], dpM[:])
            rebuild_table16(sM[:])
            sels = [mp.tile([128, 400], fp16, name=f"sel{k}") for k in range(LS_CHAINS + 1)]
            accF = mp.tile([128, 400], f32, name="accF")
            tn = mp.tile([128, 400], f32, name="tn")
            tm = mp.tile([128, T], f32, name="tm")
            for it in range(FLOW_ITERS):
                P.local_scatter(
                    sels[0][:], sTab16[:], lsI1s[:], channels=128, num_elems=400, num_idxs=VP
                )
                for k in range(LS_CHAINS):
                    P.local_scatter(
                        sels[k + 1][:], sels[k][:], lsICs[:, k * 400 : (k + 1) * 400],
                        channels=128, num_elems=400, num_idxs=400,
                    )
                Vc.tensor_add(out=accF[:], in0=sels[0][:], in1=sels[1][:])
                for k in range(2, LS_CHAINS + 1):
                    Vc.tensor_add(out=accF[:], in0=accF[:], in1=sels[k][:])
                Vc.tensor_mul(tn[:], wM[:], accF[:])
                Vc.reduce_sum(tm[:], tn[:].rearrange("p (s d) -> p s d", d=D), axis=AX.X)
                Vc.tensor_add(out=sM[:], in0=tm[:], in1=dpM[:])
                if it < FLOW_ITERS - 1:
                    rebuild_table16(sM[:])

            # s mcf-major -> node-major
            sMv = mp.tile([128, T], f32, name="sMv")
            S.dma_start(sfD[:].rearrange("(p s) o -> p (s o)", p=128), sM[:])
            S.dma_start(sMv[:].unsqueeze(2), sfD[:].rearrange("(t p) o -> p t o", p=128))

            flow = sb.tile([128, T * D], f32, name="flow")
            Vc.tensor_mul(
                flow[:].rearrange("p (t d) -> p t d", d=D),
                nW[:].rearrange("p (t d) -> p t d", d=D),
                sMv[:].unsqueeze(2).broadcast_to((128, T, D)),
            )
            Vc.tensor_mul(flow[:], flow[:], pmN[:])

            # rev gather from flow values (node-major pair-gather)
            rvf = mp.tile([128, T * D], f32, name="rvf")
            rvfo = mp.tile([128, T * D], f32, name="rvfo")
            pair_gather(flow[:], rpIdxS, rmE, rmO, rvf[:], rvfo[:])
            Vc.tensor_add(out=rvf[:], in0=rvf[:], in1=rvfo[:])
            mnf = sb.tile([128, T * D], f32, name="mnf")
            Vc.tensor_tensor(out=mnf[:], in0=flow[:], in1=rvf[:], op=ALU.min)
            Vc.tensor_sub(out=flow[:], in0=flow[:], in1=mnf[:])
            A.activation(flow[:], flow[:], AT.Relu)
            Vc.tensor_mul(flow[:], flow[:], pmN[:])

        if phase <= 4:
            S.dma_start(dbg[:, 0:400], flow[:])
            S.dma_start(out, flow[0:1, 0:1])
            ctx.close()
            return nc
        # ---------------- dual iters + costs ----------------
        with tc.tile_pool(name="dup", bufs=2) as up:
            dd01 = up.tile([128, T * D], f32, name="dd01")
            Vc.tensor_scalar_mul(out=dd01[:], in0=ddf[:], scalar1=STEP)
            fDu = up.tile([128, T * D], f32, name="fDu")
            acc = up.tile([128, T * D], f32, name="acc")
            P.memset(fDu[:], 0.0)
            P.memset(acc[:], 0.0)
            om = up.tile([128, T * D], f32, name="om")
            Vc.tensor_scalar(
                out=om[:], in0=maskf[:], scalar1=-1.0, scalar2=1.0, op0=ALU.mult, op1=ALU.add
            )
            for it in range(DUAL_ITERS):
                t2 = up.tile([128, T * D], f32, name="t2")
                Vc.scalar_tensor_tensor(
                    out=t2[:], in0=fDu[:], scalar=2.0 * STEP, in1=dd01[:], op0=ALU.mult, op1=ALU.add
                )
                Vc.scalar_tensor_tensor(
                    out=acc[:], in0=acc[:], scalar=MOM, in1=t2[:], op0=ALU.mult, op1=ALU.subtract
                )
                Vc.tensor_add(out=t2[:], in0=fDu[:], in1=acc[:])
                A.activation(t2[:], t2[:], AT.Relu)
                Vc.tensor_mul(fDu[:], t2[:], om[:])

            Vc.tensor_mul(fDu[:], fDu[:], pmN[:])
            Vc.tensor_mul(ddf[:], ddf[:], pmN[:])
            Vc.tensor_mul(dvN[:], dvN[:], pmT[:])
            junk2 = up.tile([128, T * D], f32, name="junk2")
            fc = up.tile([128, 1], f32, name="fc")
            Vc.tensor_mul(junk2[:], flow[:], flow[:])
            Vc.reduce_sum(fc[:], junk2[:], axis=AX.X)
            dtmp = up.tile([128, T * D], f32, name="dtmp")
            Vc.tensor_add(out=dtmp[:], in0=fDu[:], in1=ddf[:])
            dc = up.tile([128, 1], f32, name="dc")
            Vc.tensor_mul(junk2[:], fDu[:], dtmp[:])
            Vc.reduce_sum(dc[:], junk2[:], axis=AX.X)
            junk3 = up.tile([128, T], f32, name="junk3")
            ddem = up.tile([128, 1], f32, name="ddem")
            Vc.tensor_mul(junk3[:], dvN[:], demNs[:])
            Vc.reduce_sum(ddem[:], junk3[:], axis=AX.X)
            tot = up.tile([128, 1], f32, name="tot")
            Vc.tensor_sub(out=tot[:], in0=fc[:], in1=dc[:])
            Vc.tensor_add(out=tot[:], in0=tot[:], in1=ddem[:])
            totr = up.tile([128, 1], f32, name="totr")
            P.partition_all_reduce(totr[:], tot[:], channels=128, reduce_op=bass_isa.ReduceOp.add)
            S.dma_start(out, totr[0:1, :])
        ctx.close()
    return nc


_CACHE = {}


def _get_nc(bd2, bu2, phase=5):
    key = (round(bd2, 9), round(bu2, 9), phase)
    if key not in _CACHE:
        nc = bacc.Bacc("TRN2", target_bir_lowering=False, debug=False)
        build(nc, bd2, bu2, phase=phase)
        nc.compile()
        _CACHE[key] = nc
    return _CACHE[key]


def kernel(**inputs):
    shared = _shared_prep(inputs)
    bd2 = shared.pop("bd2")
    bu2 = shared.pop("bu2")
    nc = _get_nc(bd2, bu2)
    in_maps = []
    for b in range(B):
        core = {
            "demands": np.asarray(inputs["demands"][b]),
            "node_features": np.asarray(inputs["node_features"][b]),
            "adj_lst": np.asarray(inputs["adj_lst"][b]),
            "in_indices": np.asarray(inputs["in_indices"][b]),
            "rev_indices": np.asarray(inputs["rev_indices"][b]),
            "num_nodes": np.asarray(inputs["num_nodes"][b]),
        }
        in_maps.append(_host_prep(core, shared))
    res = run_bass_kernel_spmd(nc, in_maps, core_ids=list(range(B)))
    return np.array([res.results[b]["out"][0, 0] for b in range(B)], np.float32)


if __name__ == "__main__":
    import reference

    inputs = {k: np.asarray(v) for k, v in reference.setup_inputs().items()}
    expected = np.asarray(reference.reference(**{k: v for k, v in inputs.items()}))
    got = kernel(**inputs)
    print("expected:", expected)
    print("got:     ", got)
    err = np.abs(got - expected) / (np.abs(expected) + 1e-9)
    print("rel err: ", err.max())



# revision 22
# speedup vs baseline: 1.1542x; 1.1542x over previous
"""Bass/Trainium2 kernel for nn_AdjModel (GNN message passing).

Data-parallel over batch: 8 graphs -> 8 NeuronCores, one graph per core.
Host only reshapes/pads value tensors and precomputes integer gather-index
layouts; all math runs on device.

Layouts per core (V=5000 padded to VP=5120, T=40 node tiles of 128):
  node-major [128, T, c] : partition p, tile t  <-> node v = t*128 + p
  mcf-major  [128, 40]   : partition 16g+r, slot s <-> node u = 640g+16s+r
"""
import sys

sys.path.insert(0, "/opt/trn_rl_repo")
sys.path.insert(0, "/root/problem")

import numpy as np

import concourse.bass as bass
import concourse.bacc as bacc
import concourse.bass_isa as bass_isa
import concourse.mybir as mybir
import concourse.tile as tile
from concourse.vector_clock import ScopedClock


# ---- inlined tilefix (walrus here allows only 1 sync-wait per Drain) ----
"""Patch TileContext._drain_and_barrier: the walrus in this container only
accepts ONE sync-wait command on a Drain (CoreV3 setupSyncWait), but Tile's
final drain attaches a wait per live semaphore. Distribute the extra waits
across a chain of sync-engine NOPs placed right after the drain (same
sequencer => executed in order before the all-engine barrier + sem clears).
"""


def _patched_drain_and_barrier(self, tick_clock, wait_clock):
    nc = self.nc
    drain_inst = nc.sync.drain()
    wait_clock.add_sem_waits(drain_inst.ins, ScopedClock({None: tick_clock.global_clock}))
    si = drain_inst.ins.sync_info
    waits = list(si.on_wait) if si is not None else []
    if len(waits) > 1:
        si.on_wait = [waits[0]]
        rest = waits[1:]
        # Find any semaphore handle to seed each nop's sync_info, then
        # overwrite the wait list with the real SyncWait entries.
        assert self.sems is not None
        any_sem = next(iter(self.sems.allocated().values()))
        i = 0
        while i < len(rest):
            nop = nc.sync.nop(nofuse=True, hint="drain_wait_spill")
            nop.wait_op(any_sem, 0, "sem-ge", check=False)
            take = rest[i : i + 1]
            nop.ins.sync_info.on_wait = take
            i += 1

    nc.all_engine_barrier()
    assert self.sems is not None
    popped = nc._tile_sem_poison_stack.pop()
    assert popped is self._sem_poison
    nc.clear_and_free_semaphores(list(self.sems.allocated().values()))
    nc.all_engine_barrier()


def install():
    tile.TileContext._drain_and_barrier = _patched_drain_and_barrier


def install_ntff_hook():
    """The agent image's `antenv` lacks `axon_hooks`, so trace=True degrades.
    Recreate the module and register the ctypes NTFF hook so neuron-profile
    exec_time_ns works under axon."""
    import sys, types

    if "antenv.axon_hooks" in sys.modules:
        return
    mod = types.ModuleType("antenv.axon_hooks")
    _hook = [None]

    def set_axon_ntff_profile_hook(h):
        _hook[0] = h

    def get_axon_ntff_profile_hook():
        return _hook[0]

    mod.set_axon_ntff_profile_hook = set_axon_ntff_profile_hook
    mod.get_axon_ntff_profile_hook = get_axon_ntff_profile_hook
    sys.modules["antenv.axon_hooks"] = mod
    try:
        from trn_agent_boot.trn_boot import _ntff_profile_via_ctypes

        set_axon_ntff_profile_hook(_ntff_profile_via_ctypes("/opt/axon/libaxon_pjrt.so"))
    except Exception as e:
        print("ntff hook install failed:", e)

install()
install_ntff_hook()
from concourse.bass_utils import run_bass_kernel_spmd

mdt = mybir.dt
AT = mybir.ActivationFunctionType
ALU = mybir.AluOpType
AX = mybir.AxisListType

B, V, D = 8, 5000, 10
EMB, F, E, H = 32, 2, 64, 4
DH = E // H
VP, T = 5120, 40
LAYERS, FLOW_ITERS, DUAL_ITERS = 2, 10, 10
STEP, MOM = 0.01, 0.9
BIG = 1e9
CH = 4            # node tiles per chunk
NCHUNK = T // CH  # 10
ISQ = 1.0 / 4.0   # 1/sqrt(dh)
LS_CHAINS = 5     # local_scatter chain passes (max same-source dup depth 6)


# ---------------- host-side layout helpers ----------------

def wrap16(lst):
    """dma_gather index layout: list[i] -> sbuf[i % 16, i // 16], replicated
    across the 8 gpsimd partition groups."""
    a = np.asarray(lst, np.int16)
    n = len(a)
    assert n % 16 == 0
    a = a.reshape(n // 16, 16).T  # [16, n/16]
    return np.ascontiguousarray(np.tile(a, (8, 1)))  # [128, n/16]


def groupwrap16(lists):
    """ap_gather per-core lists: lists[g] wrapped into partitions 16g..16g+15."""
    rows = []
    for g in range(8):
        a = np.asarray(lists[g], np.int16)
        rows.append(a.reshape(len(a) // 16, 16).T)
    return np.ascontiguousarray(np.concatenate(rows, axis=0))


def node_major(a2d):
    c = a2d.shape[1]
    out = np.zeros((VP, c), a2d.dtype)
    out[: a2d.shape[0]] = a2d
    return np.ascontiguousarray(
        out.reshape(T, 128, c).transpose(1, 0, 2).reshape(128, T * c)
    )


def mcf_major(a1d):
    out = np.zeros(VP, a1d.dtype)
    out[: a1d.shape[0]] = a1d
    return np.ascontiguousarray(out.reshape(128, 40))


def edge_list_chunks(src):
    """src [VP, D] int -> flat gather list with chunk-of-CH-tiles node-major order."""
    lst = np.empty(VP * D, np.int64)
    for c in range(NCHUNK):
        base = c * CH * 128 * D
        for tt in range(CH):
            t = c * CH + tt
            for nb in range(D):
                j = tt * D + nb
                lst[base + j * 128 : base + (j + 1) * 128] = src[t * 128 : (t + 1) * 128, nb]
    return lst


def _shared_prep(inputs):
    import ml_dtypes

    f32 = np.float32

    def b16(x):
        return np.ascontiguousarray(np.asarray(x, f32).astype(ml_dtypes.bfloat16))

    emb = np.asarray(inputs["node_embedding_var"], f32)
    s = {}
    embT = np.zeros((EMB, VP), f32)
    embT[:, :V] = emb.T
    s["embT"] = embT
    s["embN"] = node_major(emb)
    W_enc = np.asarray(inputs["W_enc"], f32)
    s["Wenc1"] = np.ascontiguousarray(W_enc[:EMB])
    s["Wenc2"] = np.ascontiguousarray(W_enc[EMB:])
    s["benc"] = np.asarray(inputs["b_enc"], f32)[None]
    s["Wqkv"] = b16(np.concatenate([inputs["Wq"], inputs["Wk"], inputs["Wv"]], 1))
    s["Wo16"] = b16(inputs["Wo"])
    s["Wgru"] = b16(inputs["W_gru"])
    s["Ugru"] = b16(inputs["U_gru"])
    s["bgru"] = np.asarray(inputs["b_gru"], f32)[None]
    Wd1 = np.asarray(inputs["Wd1"], f32)
    s["Wdec"] = b16(np.concatenate([Wd1[:E], Wd1[E:], np.asarray(inputs["Wu1"], f32)], 1))
    s["bdec"] = np.concatenate(
        [np.zeros(32, f32), np.zeros(32, f32), np.asarray(inputs["bu1"], f32)]
    )[None]  # u1: no bias; y: bd1 folded into yd rows; udual: bu1
    s["Wd2rep"] = np.ascontiguousarray(np.tile(np.asarray(inputs["Wd2"], f32).T, (128, 1)))
    s["Wu2rep"] = np.ascontiguousarray(np.tile(np.asarray(inputs["Wu2"], f32).T, (128, 1)))
    s["bd2"] = float(np.asarray(inputs["bd2"]).reshape(-1)[0])
    s["bu2"] = float(np.asarray(inputs["bu2"]).reshape(-1)[0])
    bd1v = np.asarray(inputs["bd1"], f32)
    pad = np.zeros((120, 64), f32)
    pad[:, 0:32] = bd1v[None, :]
    s["ydpad120"] = pad
    s["identb"] = b16(np.eye(128, dtype=f32))
    s["ones1"] = np.ones((1, 128), f32)
    return s


def _host_prep(core, shared):
    f32 = np.float32
    import ml_dtypes

    demands = np.asarray(core["demands"], f32)[:, 0]
    feat = np.asarray(core["node_features"], f32)
    adj = np.asarray(core["adj_lst"], np.int64)
    in_idx = np.asarray(core["in_indices"], np.int64)
    rev_idx = np.asarray(core["rev_indices"], np.int64)
    num_nodes = int(core["num_nodes"])

    m = dict(shared)
    m["demN"] = node_major(demands[:, None])
    m["demM"] = mcf_major(demands)
    featT = np.zeros((F, VP), f32)
    featT[:, :V] = feat.T
    m["featT"] = featT

    maskf = (adj == num_nodes).astype(f32)
    m["maskN"] = node_major(maskf)
    m["nbigN"] = node_major(maskf * BIG)
    pm = np.zeros((VP, 1), f32)
    pm[:V] = 1.0
    m["pmaskT"] = node_major(pm)
    m["pmaskN"] = node_major(np.repeat(pm, D, axis=1))

    adjc = np.full((VP, D), V, np.int64)
    adjc[:V] = adj
    m["adjIdx"] = wrap16(edge_list_chunks(adjc))

    iv = np.zeros((VP, D), np.int64)
    iv[:V] = in_idx[..., 1]
    ip = np.zeros((VP, D), np.int64)
    ip[:V] = in_idx[..., 2]
    riv = np.zeros((VP, D), np.int64)
    riv[:V] = rev_idx[..., 1]
    rip = np.zeros((VP, D), np.int64)
    rip[:V] = rev_idx[..., 2]

    # local_scatter indices for the s-iteration: partition p owns dest nodes
    # [40p, 40p+40); edge (v, d) lives at slot (v%40)*10 + d. Pass 1 scatters
    # table position u -> first slot wanting s[u]; chain pass k copies slot of
    # use k to slot of use k+1 within the same (p, u) cell.
    v_arr = np.repeat(np.arange(V), D)
    d_arr = np.tile(np.arange(D), V)
    u_arr = iv[:V].reshape(-1)
    p_arr = v_arr // 40
    slot_arr = (v_arr % 40) * 10 + d_arr
    key = p_arr * VP + u_arr
    order = np.argsort(key, kind="stable")
    ks, sl = key[order], slot_arr[order]
    new_grp = np.r_[True, ks[1:] != ks[:-1]]
    grp_start = np.flatnonzero(new_grp)
    grp_id = np.cumsum(new_grp) - 1
    rank = np.arange(len(ks)) - grp_start[grp_id]
    assert rank.max() <= LS_CHAINS, f"dup chain depth {rank.max()} > {LS_CHAINS}"
    ls1 = np.full((128, VP), -1, np.int16)
    lsC = np.full((LS_CHAINS, 128, 400), -1, np.int16)
    m0 = rank == 0
    ls1[ks[m0] // VP, ks[m0] % VP] = sl[m0]
    for k in range(1, int(rank.max()) + 1):
        mk = rank == k
        pos = np.flatnonzero(mk)
        lsC[k - 1, ks[mk] // VP, sl[pos - 1]] = sl[mk]
    m["lsIdx1"] = ls1
    m["lsIdxC"] = np.ascontiguousarray(lsC.transpose(1, 0, 2).reshape(128, LS_CHAINS * 400))

    # pair-gather indices for the edge-weight (wM, mcf-major) and reverse-flow
    # (rvf, node-major) gathers. Flat fp16 table order e = (v%128)*400 +
    # (v//128)*10 + d (contiguous per node-major partition); ap_gather fetches
    # fp16 pairs at e>>1, parity masks select the half.
    dd = np.arange(D)[None, None, :]
    # mcf-major: partition P owns nodes 40P+s
    vv = 40 * np.arange(128)[:, None, None] + np.arange(40)[None, :, None]  # [128,40,1]
    uw = iv[vv, dd]  # [128, 40, 10]
    ew = (uw % 128) * 400 + (uw // 128) * 10 + ip[vv, dd]
    wpair = (ew >> 1).astype(np.int16)
    m["wpIdx"] = groupwrap16(
        [wpair[16 * g : 16 * g + 16].transpose(1, 0, 2).reshape(6400) for g in range(8)]
    )
    wparO = (ew & 1).astype(f32).reshape(128, 400)
    m["wParO"] = wparO
    m["wParE"] = 1.0 - wparO
    # node-major: partition p owns nodes t*128+p
    vvn = np.arange(128)[:, None, None] + 128 * np.arange(40)[None, :, None]
    ur = riv[vvn, dd]
    er = (ur % 128) * 400 + (ur // 128) * 10 + rip[vvn, dd]
    rpair = (er >> 1).astype(np.int16)
    m["rpIdx"] = groupwrap16(
        [rpair[16 * g : 16 * g + 16].transpose(1, 0, 2).reshape(6400) for g in range(8)]
    )
    rparO = (er & 1).astype(f32).reshape(128, 400)
    m["rParO"] = rparO
    m["rParE"] = 1.0 - rparO
    return m


# ---------------- device program ----------------

def build(nc, bd2, bu2, phase=5):
    f32, bf = mdt.float32, mdt.bfloat16

    def din(name, shape, dt=f32):
        return nc.dram_tensor(name, list(shape), dt, kind="ExternalInput").ap()

    embT = din("embT", [EMB, VP])
    embN = din("embN", [128, T * EMB])
    featT = din("featT", [F, VP])
    demN = din("demN", [128, T])
    demM = din("demM", [128, T])
    maskN = din("maskN", [128, T * D])
    nbigN = din("nbigN", [128, T * D])
    pmaskT = din("pmaskT", [128, T])
    pmaskN = din("pmaskN", [128, T * D])
    adjIdx = din("adjIdx", [128, 3200], mdt.int16)
    lsIdx1 = din("lsIdx1", [128, VP], mdt.int16)
    lsIdxC = din("lsIdxC", [128, LS_CHAINS * 400], mdt.int16)
    wpIdx = din("wpIdx", [128, 400], mdt.int16)
    rpIdx = din("rpIdx", [128, 400], mdt.int16)
    wParE = din("wParE", [128, 400])
    wParO = din("wParO", [128, 400])
    rParE = din("rParE", [128, 400])
    rParO = din("rParO", [128, 400])
    Wenc1 = din("Wenc1", [EMB, E])
    Wenc2 = din("Wenc2", [F, E])
    benc = din("benc", [1, E])
    Wqkv = din("Wqkv", [E, 3 * E], bf)
    Wo16 = din("Wo16", [E, E], bf)
    Wgru = din("Wgru", [E, 3 * E], bf)
    Ugru = din("Ugru", [E, 3 * E], bf)
    bgru = din("bgru", [1, 3 * E])
    Wdec = din("Wdec", [E, 96], bf)
    bdec = din("bdec", [1, 96])
    Wd2rep = din("Wd2rep", [128, 32])
    Wu2rep = din("Wu2rep", [128, 32])
    ydpad120 = din("ydpad120", [120, 64])
    identB = din("identb", [128, 128], bf)
    ones1 = din("ones1", [1, 128])

    out = nc.dram_tensor("out", [1, 1], f32, kind="ExternalOutput").ap()
    dbg = nc.dram_tensor("dbg", [128, 2560], f32, kind="ExternalOutput").ap()

    with tile.TileContext(nc) as tc:
        import contextlib

        ctx = contextlib.ExitStack()
        sb = ctx.enter_context(tc.tile_pool(name="sb", bufs=1))
        dram = ctx.enter_context(tc.tile_pool(name="dram", bufs=1, space="DRAM"))
        midctx = contextlib.ExitStack()
        mid = midctx.enter_context(tc.tile_pool(name="mid", bufs=1))

        S, A, Vc, P, PE = nc.sync, nc.scalar, nc.vector, nc.gpsimd, nc.tensor

        # persistent state
        x = mid.tile([128, T * E], f32, name="x")
        u1 = mid.tile([128, T * 32], f32, name="u1")
        xT = mid.tile([E, VP], bf, name="xT")
        q = mid.tile([128, T * E], bf, name="q")
        maskf = sb.tile([128, T * D], f32, name="maskf")
        nbig = sb.tile([128, T * D], f32, name="nbig")
        pmT = sb.tile([128, T], f32, name="pmT")
        pmN = sb.tile([128, T * D], f32, name="pmN")
        zkv = mid.tile([128, 128], bf, name="zkv")
        onesb = sb.tile([1, 128], f32, name="onesb")
        identb = mid.tile([128, 128], bf, name="identb_sb")
        adjI = mid.tile([128, 3200], mdt.int16, name="adjI")
        nW = sb.tile([128, T * D], f32, name="nW")
        dvN = sb.tile([128, T], f32, name="dvN")
        ddf = sb.tile([128, T * D], f32, name="ddf")
        dp = sb.tile([128, T], f32, name="dp")
        dpM = sb.tile([128, T], f32, name="dpM")
        demNs = sb.tile([128, T], f32, name="demNs")
        wq16 = sb.tile([E, 3 * E], bf, name="wq16")
        wo16s = sb.tile([E, E], bf, name="wo16s")
        wg16 = sb.tile([E, 3 * E], bf, name="wg16")
        ug16 = sb.tile([E, 3 * E], bf, name="ug16")
        wd16 = sb.tile([E, 96], bf, name="wd16")
        bg = sb.tile([1, 3 * E], f32, name="bg")
        bdc = sb.tile([1, 96], f32, name="bdc")
        wd2r = sb.tile([128, 32], f32, name="wd2r")
        wu2r = sb.tile([128, 32], f32, name="wu2r")

        kvDs = [dram.tile([VP, 128], bf, name=f"kvD{l}") for l in range(LAYERS)]
        ydD = dram.tile([VP, E], f32, name="ydD")
        sfD = dram.tile([VP, 1], f32, name="sfD")
        sfD16 = dram.tile([VP, 1], mdt.float16, name="sfD16")
        wpD = dram.tile([VP * D, 1], mdt.float16, name="wpD")

        nidxreg = P.alloc_register("nidxreg")
        P.reg_mov(nidxreg, CH * D * 128)
        S.dma_start(onesb[:], ones1)
        S.dma_start(identb[:], identB)
        S.dma_start(adjI[:], adjIdx)
        S.dma_start(maskf[:], maskN)
        S.dma_start(nbig[:], nbigN)
        S.dma_start(pmT[:], pmaskT)
        S.dma_start(pmN[:], pmaskN)
        P.memset(zkv[:], 0.0)
        S.dma_start(wq16[:], Wqkv)
        S.dma_start(wo16s[:], Wo16)
        S.dma_start(wg16[:], Wgru)
        S.dma_start(ug16[:], Ugru)
        S.dma_start(bg[:], bgru)
        S.dma_start(wd16[:], Wdec)
        S.dma_start(bdc[:], bdec)
        S.dma_start(wd2r[:], Wd2rep)
        S.dma_start(wu2r[:], Wu2rep)
        S.dma_start(demNs[:], demN)

        # ---------------- encode ----------------
        with tc.tile_pool(name="encp", bufs=2) as enc, tc.tile_pool(
            name="encps", bufs=2, space="PSUM"
        ) as eps:
            embTs = enc.tile([EMB, VP], f32, name="embTs")
            featTs = enc.tile([F, VP], f32, name="featTs")
            embNs = enc.tile([128, T * EMB], f32, name="embNs")
            w1 = enc.tile([EMB, E], f32, name="w1")
            w2 = enc.tile([F, E], f32, name="w2")
            be = enc.tile([1, E], f32, name="be")
            demMs = enc.tile([128, T], f32, name="demMs")
            S.dma_start(embTs[:], embT)
            S.dma_start(featTs[:], featT)
            S.dma_start(embNs[:], embN)
            S.dma_start(w1[:], Wenc1)
            S.dma_start(w2[:], Wenc2)
            S.dma_start(be[:], benc)
            S.dma_start(demMs[:], demM)

            A.activation(dp[:], demNs[:], AT.Relu)
            A.activation(dpM[:], demMs[:], AT.Relu)

            sqv = enc.tile([128, T * EMB], f32, name="sqv")
            Vc.tensor_mul(sqv[:], embNs[:], embNs[:])
            n2 = enc.tile([128, T], f32, name="n2")
            Vc.reduce_sum(n2[:], sqv[:].rearrange("p (t c) -> p t c", c=EMB), axis=AX.X)
            nrm = enc.tile([128, T], f32, name="nrm")
            A.activation(nrm[:], n2[:], AT.Sqrt)
            Vc.tensor_scalar_max(out=nrm[:], in0=nrm[:], scalar1=1.0)
            scl = enc.tile([128, T], f32, name="scl")
            Vc.reciprocal(scl[:], nrm[:])

            for t in range(T):
                p1 = eps.tile([128, E], f32, name="p1")
                p2 = eps.tile([128, E], f32, name="p2")
                PE.matmul(p1[:], embTs[:, t * 128 : (t + 1) * 128], w1[:], start=True, stop=True)
                PE.matmul(p2[:], featTs[:, t * 128 : (t + 1) * 128], w2[:], start=True, stop=False)
                PE.matmul(p2[:], onesb[:], be[:], start=False, stop=True)
                A.activation(x[:, t * E : (t + 1) * E], p2[:], AT.Copy)
                Vc.scalar_tensor_tensor(
                    out=x[:, t * E : (t + 1) * E], in0=p1[:], scalar=scl[:, t : t + 1],
                    in1=x[:, t * E : (t + 1) * E], op0=ALU.mult, op1=ALU.add,
                )
                xb = enc.tile([128, E], bf, name="xb")
                A.activation(xb[:], x[:, t * E : (t + 1) * E], AT.Copy)
                xtp = eps.tile([E, 128], bf, name="xtp")
                PE.transpose(xtp[:], xb[:], identb[:])
                A.activation(xT[:, t * 128 : (t + 1) * 128], xtp[:], AT.Copy)

        if phase <= 1:
            S.dma_start(dbg[:], x[:])
            S.dma_start(out, x[0:1, 0:1])
            midctx.close()
            ctx.close()
            return nc
        # ---------------- graph layers ----------------
        layctx = contextlib.ExitStack()
        kvps = layctx.enter_context(tc.tile_pool(name="kvps", bufs=3, space="PSUM"))
        for layer in range(LAYERS):
            with tc.tile_pool(name=f"lay{layer}", bufs=4) as lp, tc.tile_pool(
                name=f"lps{layer}", bufs=1, space="PSUM"
            ) as lps:
                kvD = kvDs[layer]

                def kv_chunk(c, dstD):
                    kvc = lp.tile([128, CH, 128], bf, name="kvc")
                    for tt in range(CH):
                        t = c * CH + tt
                        pq = kvps.tile([128, 3 * E], f32, name="pq")
                        PE.matmul(pq[:], xT[:, t * 128 : (t + 1) * 128], wq16[:], start=True, stop=True)
                        A.activation(q[:, t * E : (t + 1) * E], pq[:, :E], AT.Copy)
                        A.activation(kvc[:, tt, :], pq[:, E:], AT.Copy)
                    S.dma_start(
                        dstD[:].rearrange("(t p) c -> p t c", p=128)[:, c * CH : (c + 1) * CH, :],
                        kvc[:],
                    )

                for c in range(NCHUNK):
                    kv_chunk(c, kvD)
                S.dma_start(
                    kvD[:].rearrange("(t p) c -> p t c", p=128)[8:128, T - 1, :],
                    zkv[8:128, :],
                )

                for c in range(NCHUNK):
                    kvn = lp.tile([128, CH * D, 128], bf, name="kvn")
                    P.dma_gather(
                        kvn[:], kvD[:], adjI[:, c * 320 : (c + 1) * 320],
                        num_idxs=CH * D * 128, num_idxs_reg=nidxreg, elem_size=128, single_packet=False,
                    )
                    # scores
                    prodk = lp.tile([128, CH * D * E], bf, name="prodk")
                    qv = (
                        q[:, c * CH * E : (c + 1) * CH * E]
                        .rearrange("p (tt e) -> p tt e", e=E)
                        .unsqueeze(2)
                        .broadcast_to((128, CH, D, E))
                    )
                    Vc.tensor_mul(
                        prodk[:].rearrange("p (tt nb e) -> p tt nb e", nb=D, e=E),
                        kvn[:, :, 0:E].rearrange("p (tt nb) e -> p tt nb e", nb=D),
                        qv,
                    )
                    sc = lp.tile([128, CH * D * H], f32, name="sc")
                    Vc.reduce_sum(
                        sc[:].rearrange("p (tn h) -> p tn h", h=H),
                        prodk[:].rearrange("p (tnh dh) -> p tnh dh", dh=DH),
                        axis=AX.X,
                    )
                    scm = lp.tile([128, CH * D * H], f32, name="scm")
                    Vc.scalar_tensor_tensor(
                        out=scm[:].rearrange("p (tn h) -> p tn h", h=H),
                        in0=sc[:].rearrange("p (tn h) -> p tn h", h=H),
                        scalar=ISQ,
                        in1=nbig[:, c * CH * D : (c + 1) * CH * D].unsqueeze(2).broadcast_to((128, CH * D, H)),
                        op0=ALU.mult, op1=ALU.subtract,
                    )
                    ex = lp.tile([128, CH * D * H], f32, name="ex")
                    A.activation(ex[:], scm[:], AT.Exp)
                    zs = lp.tile([128, CH * H], f32, name="zs")
                    Vc.reduce_sum(
                        zs[:].rearrange("p (tt h) -> p tt h", h=H),
                        ex[:].rearrange("p (tt nb h) -> p tt h nb", nb=D, h=H),
                        axis=AX.X,
                    )
                    rz = lp.tile([128, CH * H], f32, name="rz")
                    Vc.reciprocal(rz[:], zs[:])
                    at = lp.tile([128, CH * D * H], f32, name="at")
                    Vc.tensor_mul(
                        at[:].rearrange("p (tt nb h) -> p tt nb h", nb=D, h=H),
                        ex[:].rearrange("p (tt nb h) -> p tt nb h", nb=D, h=H),
                        rz[:].rearrange("p (tt h) -> p tt h", h=H).unsqueeze(2).broadcast_to((128, CH, D, H)),
                    )
                    prodv = lp.tile([128, CH * D * E], f32, name="prodv")
                    Vc.tensor_mul(
                        prodv[:].rearrange("p (tt nb h dh) -> p tt nb h dh", nb=D, h=H, dh=DH),
                        kvn[:, :, E:].rearrange("p (tt nb) (h dh) -> p tt nb h dh", nb=D, h=H),
                        at[:].rearrange("p (tt nb h) -> p tt nb h", nb=D, h=H).unsqueeze(4).broadcast_to(
                            (128, CH, D, H, DH)
                        ),
                    )
                    agg = lp.tile([128, CH * E], f32, name="agg")
                    Vc.reduce_sum(
                        agg[:].rearrange("p (tt e) -> p tt e", e=E),
                        prodv[:].rearrange("p (tt nb e) -> p tt e nb", nb=D, e=E),
                        axis=AX.X,
                    )
                    # GRU per tile
                    for tt in range(CH):
                        t = c * CH + tt
                        aggb = lp.tile([128, E], bf, name="aggb")
                        A.activation(aggb[:], agg[:, tt * E : (tt + 1) * E], AT.Copy)
                        agT = lps.tile([E, 128], bf, name="trT")
                        PE.transpose(agT[:], aggb[:], identb[:])
                        agTs = lp.tile([E, 128], bf, name="agTs")
                        A.activation(agTs[:], agT[:], AT.Copy)
                        pnx = lps.tile([128, E], f32, name="pnx")
                        PE.matmul(pnx[:], agTs[:], wo16s[:], start=True, stop=True)
                        nxt = lp.tile([128, E], bf, name="nxt")
                        A.activation(nxt[:], pnx[:], AT.Tanh)
                        nxT = lps.tile([E, 128], bf, name="trT")
                        PE.transpose(nxT[:], nxt[:], identb[:])
                        nxTs = lp.tile([E, 128], bf, name="nxTs")
                        A.activation(nxTs[:], nxT[:], AT.Copy)
                        pA = lps.tile([128, 2 * E], f32, name="pA")
                        PE.matmul(pA[:], nxTs[:], wg16[:, : 2 * E], start=True, stop=False)
                        PE.matmul(pA[:], xT[:, t * 128 : (t + 1) * 128], ug16[:, : 2 * E], start=False, stop=False)
                        PE.matmul(pA[:], onesb[:], bg[:, : 2 * E], start=False, stop=True)
                        pBC = lps.tile([128, 2 * E], f32, name="pBC")
                        PE.matmul(pBC[:, :E], nxTs[:], wg16[:, 2 * E :], start=True, stop=False)
                        PE.matmul(pBC[:, :E], onesb[:], bg[:, 2 * E :], start=False, stop=True)
                        PE.matmul(pBC[:, E:], xT[:, t * 128 : (t + 1) * 128], ug16[:, 2 * E :], start=True, stop=True)
                        zr = lp.tile([128, 2 * E], f32, name="zr")
                        A.activation(zr[:], pA[:], AT.Sigmoid)
                        tmp = lp.tile([128, E], f32, name="tmp")
                        Vc.tensor_mul(tmp[:], zr[:, E:], pBC[:, E:])
                        Vc.tensor_add(out=tmp[:], in0=tmp[:], in1=pBC[:, :E])
                        hh = lp.tile([128, E], f32, name="hh")
                        A.activation(hh[:], tmp[:], AT.Tanh)
                        hmx = lp.tile([128, E], f32, name="hmx")
                        Vc.tensor_sub(out=hmx[:], in0=hh[:], in1=x[:, t * E : (t + 1) * E])
                        Vc.tensor_mul(hmx[:], hmx[:], zr[:, :E])
                        Vc.tensor_add(
                            out=x[:, t * E : (t + 1) * E], in0=x[:, t * E : (t + 1) * E], in1=hmx[:]
                        )
                        xb2 = lp.tile([128, E], bf, name="xb2")
                        A.activation(xb2[:], x[:, t * E : (t + 1) * E], AT.Copy)
                        xtp2 = lps.tile([E, 128], bf, name="xtp2")
                        PE.transpose(xtp2[:], xb2[:], identb[:])
                        A.activation(xT[:, t * 128 : (t + 1) * 128], xtp2[:], AT.Copy)

        layctx.close()
        if phase <= 2:
            S.dma_start(dbg[:], x[:])
            S.dma_start(out, x[0:1, 0:1])
            midctx.close()
            ctx.close()
            return nc
        # ---------------- decoders ----------------
        with tc.tile_pool(name="decp", bufs=3) as dpool, tc.tile_pool(
            name="decps", bufs=2, space="PSUM"
        ) as dps:
            for c in range(NCHUNK):
                ydc = dpool.tile([128, CH, 33], f32, name="ydc")
                for tt in range(CH):
                    t = c * CH + tt
                    pd = dps.tile([128, 96], f32, name="pd")
                    PE.matmul(pd[:], xT[:, t * 128 : (t + 1) * 128], wd16[:], start=True, stop=False)
                    PE.matmul(pd[:], onesb[:], bdc[:], start=False, stop=True)
                    Vc.tensor_copy(u1[:, t * 32 : (t + 1) * 32], pd[:, 0:32])
                    Vc.tensor_copy(ydc[:, tt, 0:32], pd[:, 32:64])
                    th = dpool.tile([128, 32], f32, name="th")
                    A.activation(th[:], pd[:, 64:96], AT.Tanh)
                    junk = dpool.tile([128, 32], f32, name="junk")
                    Vc.tensor_mul(junk[:], th[:], wu2r[:])
                    Vc.reduce_sum(dvN[:, t : t + 1], junk[:], axis=AX.X)
                    Vc.tensor_scalar_add(out=dvN[:, t : t + 1], in0=dvN[:, t : t + 1], scalar1=bu2)
                    Vc.tensor_copy(ydc[:, tt, 32:33], dvN[:, t : t + 1])
                S.dma_start(
                    ydD[:].rearrange("(t p) c -> p t c", p=128)[:, c * CH : (c + 1) * CH, 0:33],
                    ydc[:],
                )
            # pad rows 5000..5119 = [bd1 | 0]
            ydp = dpool.tile([120, 64], f32, name="ydp")
            S.dma_start(ydp[:], ydpad120)
            S.dma_start(ydD[:].rearrange("(t p) c -> p t c", p=128)[8:128, T - 1, :], ydp[:])

            for c in range(NCHUNK):
                ydg = dpool.tile([128, CH * D, E], f32, name="ydg")
                P.dma_gather(
                    ydg[:], ydD[:], adjI[:, c * 320 : (c + 1) * 320],
                    num_idxs=CH * D * 128, num_idxs_reg=nidxreg, elem_size=E, single_packet=False,
                )
                h1p = dpool.tile([128, CH * D * 32], f32, name="h1p")
                u1v = (
                    u1[:, c * CH * 32 : (c + 1) * CH * 32]
                    .rearrange("p (tt k) -> p tt k", k=32)
                    .unsqueeze(2)
                    .broadcast_to((128, CH, D, 32))
                )
                h1p4 = h1p[:].rearrange("p (tt nb k) -> p tt nb k", nb=D, k=32)
                mk4 = (
                    maskf[:, c * CH * D : (c + 1) * CH * D]
                    .rearrange("p (tt nb) -> p tt nb", nb=D)
                    .unsqueeze(3)
                    .broadcast_to((128, CH, D, 32))
                )
                # h1p = u1*maskf ; then u1 - u1*maskf ; then + ydg
                Vc.tensor_mul(h1p4, u1v, mk4)
                Vc.tensor_sub(out=h1p4, in0=u1v, in1=h1p4)
                Vc.tensor_add(
                    out=h1p4,
                    in0=h1p4,
                    in1=ydg[:, :, 0:32].rearrange("p (tt nb) k -> p tt nb k", nb=D),
                )
                h1t = dpool.tile([128, CH * D * 32], f32, name="h1t")
                A.activation(h1t[:], h1p[:], AT.Tanh)
                pw = dpool.tile([128, CH * D * 32], f32, name="pw")
                Vc.tensor_mul(
                    pw[:].rearrange("p (tn k) -> p tn k", k=32),
                    h1t[:].rearrange("p (tn k) -> p tn k", k=32),
                    wd2r[:].unsqueeze(1).broadcast_to((128, CH * D, 32)),
                )
                nwc = dpool.tile([128, CH * D], f32, name="nwc")
                Vc.reduce_sum(nwc[:], pw[:].rearrange("p (tn k) -> p tn k", k=32), axis=AX.X)
                prd = dpool.tile([128, CH * D], f32, name="prd")
                Vc.tensor_sub(out=prd[:], in0=nwc[:], in1=nbig[:, c * CH * D : (c + 1) * CH * D])
                exn = dpool.tile([128, CH * D], f32, name="exn")
                A.activation(exn[:], prd[:], AT.Exp)
                zn = dpool.tile([128, CH], f32, name="zn")
                Vc.reduce_sum(zn[:], exn[:].rearrange("p (tt nb) -> p tt nb", nb=D), axis=AX.X)
                Vc.tensor_scalar_add(out=zn[:], in0=zn[:], scalar1=1e-30)
                rzn = dpool.tile([128, CH], f32, name="rzn")
                Vc.reciprocal(rzn[:], zn[:])
                Vc.tensor_mul(
                    nW[:, c * CH * D : (c + 1) * CH * D].rearrange("p (tt nb) -> p tt nb", nb=D),
                    exn[:].rearrange("p (tt nb) -> p tt nb", nb=D),
                    rzn[:].unsqueeze(2).broadcast_to((128, CH, D)),
                )
                # dual_diff = ydg[:, :, 32] - maskf*dv
                Vc.tensor_mul(
                    ddf[:, c * CH * D : (c + 1) * CH * D].rearrange("p (tt nb) -> p tt nb", nb=D),
                    maskf[:, c * CH * D : (c + 1) * CH * D].rearrange("p (tt nb) -> p tt nb", nb=D),
                    dvN[:, c * CH : (c + 1) * CH].unsqueeze(2).broadcast_to((128, CH, D)),
                )
                Vc.tensor_sub(
                    out=ddf[:, c * CH * D : (c + 1) * CH * D],
                    in0=ydg[:, :, 32],
                    in1=ddf[:, c * CH * D : (c + 1) * CH * D],
                )

        if phase <= 3:
            S.dma_start(dbg[:, 0:400], nW[:])
            S.dma_start(dbg[:, 400:800], ddf[:])
            S.dma_start(out, nW[0:1, 0:1])
            midctx.close()
            ctx.close()
            return nc
        # ---------------- mcf flow ----------------
        midctx.close()
        with tc.tile_pool(name="mcfp", bufs=1) as mp, tc.tile_pool(
            name="mcfps", bufs=2, space="PSUM"
        ) as mps:
            fp16 = mdt.float16
            from concourse.bass import _add_dep_helper

            lsI1s = mp.tile([128, VP], mdt.int16, name="lsI1s")
            S.dma_start(lsI1s[:], lsIdx1)
            lsICs = mp.tile([128, LS_CHAINS * 400], mdt.int16, name="lsICs")
            S.dma_start(lsICs[:], lsIdxC)
            wpIdxS = mp.tile([128, 400], mdt.int16, name="wpIdxS")
            S.dma_start(wpIdxS[:], wpIdx)
            rpIdxS = mp.tile([128, 400], mdt.int16, name="rpIdxS")
            S.dma_start(rpIdxS[:], rpIdx)
            wmE = mp.tile([128, 400], f32, name="wmE")
            S.dma_start(wmE[:], wParE)
            wmO = mp.tile([128, 400], f32, name="wmO")
            S.dma_start(wmO[:], wParO)
            rmE = mp.tile([128, 400], f32, name="rmE")
            S.dma_start(rmE[:], rParE)
            rmO = mp.tile([128, 400], f32, name="rmO")
            S.dma_start(rmO[:], rParO)

            wTab = mp.tile([128, VP * D], fp16, name="wTab")
            ev16 = mp.tile([128, 400], fp16, name="ev16")
            gpair = mp.tile([128, 12800], fp16, name="gpair")
            psel = mp.tile([128, 800], fp16, name="psel")

            def pair_gather(vals_f32, idx_tile, mE, mO, outA, outB):
                """flat fp16 table[e]: outA = even-half*mE, outB = odd-half*mO;
                caller sums. vals_f32 [128, 400] node-major edge values."""
                A.activation(ev16[:], vals_f32, AT.Copy)
                S.dma_start(wpD[:].rearrange("(p c) o -> p (c o)", p=128), ev16[:])
                S.dma_start(
                    wTab[:],
                    wpD[:].rearrange("(a v) o -> a (v o)", a=1).broadcast_to((128, VP * D)),
                )
                apg = P.ap_gather(
                    gpair[:], wTab[:], idx_tile[:],
                    channels=128, num_elems=VP * D // 2, d=2, num_idxs=6400,
                )
                for r in range(16):
                    eng = S if r % 2 == 0 else A
                    dmi = eng.dma_start(
                        psel[r:128:16, :].rearrange("p (s d c) -> p s d c", d=D, c=2),
                        gpair[r:128:16, :].rearrange(
                            "p (s r2 d c) -> p s r2 d c", r2=16, d=D, c=2
                        )[:, :, r, :, :],
                    )
                    _add_dep_helper(dmi.ins, apg.ins, sync=True, reason="extract")
                pv = psel[:].rearrange("p (s c) -> p s c", c=2)
                Vc.tensor_mul(outA, pv[:, :, 0], mE[:])
                Vc.tensor_mul(outB, pv[:, :, 1], mO[:])

            wM = mp.tile([128, 400], f32, name="wM")
            wMo = mp.tile([128, 400], f32, name="wMo")
            pair_gather(nW[:], wpIdxS, wmE, wmO, wM[:], wMo[:])
            Vc.tensor_add(out=wM[:], in0=wM[:], in1=wMo[:])
            fp16 = mdt.float16
            sTab16 = mp.tile([128, VP], fp16, name="sTab16")
            sM16 = mp.tile([128, T], fp16, name="sM16")
            sM = mp.tile([128, T], f32, name="sM")

            def rebuild_table16(src_mcf):
                A.activation(sM16[:], src_mcf, AT.Copy)
                S.dma_start(sfD16[:].rearrange("(p s) o -> p (s o)", p=128), sM16[:])
                S.dma_start(
                    sTab16[:],
                    sfD16[:].rearrange("(a v) o -> a (v o)", a=1).broadcast_to((128, VP)),
                )

            Vc.tensor_copy(sM[:], dpM[:])
            rebuild_table16(sM[:])
            sels = [mp.tile([128, 400], fp16, name=f"sel{k}") for k in range(LS_CHAINS + 1)]
            accF = mp.tile([128, 400], f32, name="accF")
            tn = mp.tile([128, 400], f32, name="tn")
            tm = mp.tile([128, T], f32, name="tm")
            for it in range(FLOW_ITERS):
                P.local_scatter(
                    sels[0][:], sTab16[:], lsI1s[:], channels=128, num_elems=400, num_idxs=VP
                )
                for k in range(LS_CHAINS):
                    P.local_scatter(
                        sels[k + 1][:], sels[k][:], lsICs[:, k * 400 : (k + 1) * 400],
                        channels=128, num_elems=400, num_idxs=400,
                    )
                Vc.tensor_add(out=accF[:], in0=sels[0][:], in1=sels[1][:])
                for k in range(2, LS_CHAINS + 1):
                    Vc.tensor_add(out=accF[:], in0=accF[:], in1=sels[k][:])
                Vc.tensor_mul(tn[:], wM[:], accF[:])
                Vc.reduce_sum(tm[:], tn[:].rearrange("p (s d) -> p s d", d=D), axis=AX.X)
                Vc.tensor_add(out=sM[:], in0=tm[:], in1=dpM[:])
                if it < FLOW_ITERS - 1:
                    rebuild_table16(sM[:])

            # s mcf-major -> node-major
            sMv = mp.tile([128, T], f32, name="sMv")
            S.dma_start(sfD[:].rearrange("(p s) o -> p (s o)", p=128), sM[:])
            S.dma_start(sMv[:].unsqueeze(2), sfD[:].rearrange("(t p) o -> p t o", p=128))

            flow = sb.tile([128, T * D], f32, name="flow")
            Vc.tensor_mul(
                flow[:].rearrange("p (t d) -> p t d", d=D),
                nW[:].rearrange("p (t d) -> p t d", d=D),
                sMv[:].unsqueeze(2).broadcast_to((128, T, D)),
            )
            Vc.tensor_mul(flow[:], flow[:], pmN[:])

            # rev gather from flow values (node-major pair-gather)
            rvf = mp.tile([128, T * D], f32, name="rvf")
            rvfo = mp.tile([128, T * D], f32, name="rvfo")
            pair_gather(flow[:], rpIdxS, rmE, rmO, rvf[:], rvfo[:])
            Vc.tensor_add(out=rvf[:], in0=rvf[:], in1=rvfo[:])
            mnf = sb.tile([128, T * D], f32, name="mnf")
            Vc.tensor_tensor(out=mnf[:], in0=flow[:], in1=rvf[:], op=ALU.min)
            Vc.tensor_sub(out=flow[:], in0=flow[:], in1=mnf[:])
            A.activation(flow[:], flow[:], AT.Relu)
            Vc.tensor_mul(flow[:], flow[:], pmN[:])

        if phase <= 4:
            S.dma_start(dbg[:, 0:400], flow[:])
            S.dma_start(out, flow[0:1, 0:1])
            ctx.close()
            return nc
        # ---------------- dual iters + costs ----------------
        with tc.tile_pool(name="dup", bufs=2) as up:
            dd01 = up.tile([128, T * D], f32, name="dd01")
            Vc.tensor_scalar_mul(out=dd01[:], in0=ddf[:], scalar1=STEP)
            fDu = up.tile([128, T * D], f32, name="fDu")
            acc = up.tile([128, T * D], f32, name="acc")
            P.memset(fDu[:], 0.0)
            P.memset(acc[:], 0.0)
            om = up.tile([128, T * D], f32, name="om")
            Vc.tensor_scalar(
                out=om[:], in0=maskf[:], scalar1=-1.0, scalar2=1.0, op0=ALU.mult, op1=ALU.add
            )
            for it in range(DUAL_ITERS):
                t2 = up.tile([128, T * D], f32, name="t2")
                Vc.scalar_tensor_tensor(
                    out=t2[:], in0=fDu[:], scalar=2.0 * STEP, in1=dd01[:], op0=ALU.mult, op1=ALU.add
                )
                Vc.scalar_tensor_tensor(
                    out=acc[:], in0=acc[:], scalar=MOM, in1=t2[:], op0=ALU.mult, op1=ALU.subtract
                )
                Vc.tensor_add(out=t2[:], in0=fDu[:], in1=acc[:])
                A.activation(t2[:], t2[:], AT.Relu)
                Vc.tensor_mul(fDu[:], t2[:], om[:])

            Vc.tensor_mul(fDu[:], fDu[:], pmN[:])
            Vc.tensor_mul(ddf[:], ddf[:], pmN[:])
            Vc.tensor_mul(dvN[:], dvN[:], pmT[:])
            junk2 = up.tile([128, T * D], f32, name="junk2")
            fc = up.tile([128, 1], f32, name="fc")
            Vc.tensor_mul(junk2[:], flow[:], flow[:])
            Vc.reduce_sum(fc[:], junk2[:], axis=AX.X)
            dtmp = up.tile([128, T * D], f32, name="dtmp")
            Vc.tensor_add(out=dtmp[:], in0=fDu[:], in1=ddf[:])
            dc = up.tile([128, 1], f32, name="dc")
            Vc.tensor_mul(junk2[:], fDu[:], dtmp[:])
            Vc.reduce_sum(dc[:], junk2[:], axis=AX.X)
            junk3 = up.tile([128, T], f32, name="junk3")
            ddem = up.tile([128, 1], f32, name="ddem")
            Vc.tensor_mul(junk3[:], dvN[:], demNs[:])
            Vc.reduce_sum(ddem[:], junk3[:], axis=AX.X)
            tot = up.tile([128, 1], f32, name="tot")
            Vc.tensor_sub(out=tot[:], in0=fc[:], in1=dc[:])
            Vc.tensor_add(out=tot[:], in0=tot[:], in1=ddem[:])
            totr = up.tile([128, 1], f32, name="totr")
            P.partition_all_reduce(totr[:], tot[:], channels=128, reduce_op=bass_isa.ReduceOp.add)
            S.dma_start(out, totr[0:1, :])
        ctx.close()
    return nc


_CACHE = {}


def _get_nc(bd2, bu2, phase=5):
    key = (round(bd2, 9), round(bu2, 9), phase)
    if key not in _CACHE:
        nc = bacc.Bacc("TRN2", target_bir_lowering=False, debug=False)
        build(nc, bd2, bu2, phase=phase)
        nc.compile()
        _CACHE[key] = nc
    return _CACHE[key]


def kernel(**inputs):
    shared = _shared_prep(inputs)
    bd2 = shared.pop("bd2")
    bu2 = shared.pop("bu2")
    nc = _get_nc(bd2, bu2)
    in_maps = []
    for b in range(B):
        core = {
            "demands": np.asarray(inputs["demands"][b]),
            "node_features": np.asarray(inputs["node_features"][b]),
            "adj_lst": np.asarray(inputs["adj_lst"][b]),
            "in_indices": np.asarray(inputs["in_indices"][b]),
            "rev_indices": np.asarray(inputs["rev_indices"][b]),
            "num_nodes": np.asarray(inputs["num_nodes"][b]),
        }
        in_maps.append(_host_prep(core, shared))
    res = run_bass_kernel_spmd(nc, in_maps, core_ids=list(range(B)))
    return np.array([res.results[b]["out"][0, 0] for b in range(B)], np.float32)


if __name__ == "__main__":
    import reference

    inputs = {k: np.asarray(v) for k, v in reference.setup_inputs().items()}
    expected = np.asarray(reference.reference(**{k: v for k, v in inputs.items()}))
    got = kernel(**inputs)
    print("expected:", expected)
    print("got:     ", got)
    err = np.abs(got - expected) / (np.abs(expected) + 1e-9)
    print("rel err: ", err.max())



# revision 24
# speedup vs baseline: 1.1756x; 1.0185x over previous
"""Bass/Trainium2 kernel for nn_AdjModel (GNN message passing).

Data-parallel over batch: 8 graphs -> 8 NeuronCores, one graph per core.
Host only reshapes/pads value tensors and precomputes integer gather-index
layouts; all math runs on device.

Layouts per core (V=5000 padded to VP=5120, T=40 node tiles of 128):
  node-major [128, T, c] : partition p, tile t  <-> node v = t*128 + p
  mcf-major  [128, 40]   : partition 16g+r, slot s <-> node u = 640g+16s+r
"""
import sys

sys.path.insert(0, "/opt/trn_rl_repo")
sys.path.insert(0, "/root/problem")

import numpy as np

import concourse.bass as bass
import concourse.bacc as bacc
import concourse.bass_isa as bass_isa
import concourse.mybir as mybir
import concourse.tile as tile
from concourse.vector_clock import ScopedClock


# ---- inlined tilefix (walrus here allows only 1 sync-wait per Drain) ----
"""Patch TileContext._drain_and_barrier: the walrus in this container only
accepts ONE sync-wait command on a Drain (CoreV3 setupSyncWait), but Tile's
final drain attaches a wait per live semaphore. Distribute the extra waits
across a chain of sync-engine NOPs placed right after the drain (same
sequencer => executed in order before the all-engine barrier + sem clears).
"""


def _patched_drain_and_barrier(self, tick_clock, wait_clock):
    nc = self.nc
    drain_inst = nc.sync.drain()
    wait_clock.add_sem_waits(drain_inst.ins, ScopedClock({None: tick_clock.global_clock}))
    si = drain_inst.ins.sync_info
    waits = list(si.on_wait) if si is not None else []
    if len(waits) > 1:
        si.on_wait = [waits[0]]
        rest = waits[1:]
        # Find any semaphore handle to seed each nop's sync_info, then
        # overwrite the wait list with the real SyncWait entries.
        assert self.sems is not None
        any_sem = next(iter(self.sems.allocated().values()))
        i = 0
        while i < len(rest):
            nop = nc.sync.nop(nofuse=True, hint="drain_wait_spill")
            nop.wait_op(any_sem, 0, "sem-ge", check=False)
            take = rest[i : i + 1]
            nop.ins.sync_info.on_wait = take
            i += 1

    nc.all_engine_barrier()
    assert self.sems is not None
    popped = nc._tile_sem_poison_stack.pop()
    assert popped is self._sem_poison
    nc.clear_and_free_semaphores(list(self.sems.allocated().values()))
    nc.all_engine_barrier()


def install():
    tile.TileContext._drain_and_barrier = _patched_drain_and_barrier


def install_ntff_hook():
    """The agent image's `antenv` lacks `axon_hooks`, so trace=True degrades.
    Recreate the module and register the ctypes NTFF hook so neuron-profile
    exec_time_ns works under axon."""
    import sys, types

    if "antenv.axon_hooks" in sys.modules:
        return
    mod = types.ModuleType("antenv.axon_hooks")
    _hook = [None]

    def set_axon_ntff_profile_hook(h):
        _hook[0] = h

    def get_axon_ntff_profile_hook():
        return _hook[0]

    mod.set_axon_ntff_profile_hook = set_axon_ntff_profile_hook
    mod.get_axon_ntff_profile_hook = get_axon_ntff_profile_hook
    sys.modules["antenv.axon_hooks"] = mod
    try:
        from trn_agent_boot.trn_boot import _ntff_profile_via_ctypes

        set_axon_ntff_profile_hook(_ntff_profile_via_ctypes("/opt/axon/libaxon_pjrt.so"))
    except Exception as e:
        print("ntff hook install failed:", e)

install()
install_ntff_hook()
from concourse.bass_utils import run_bass_kernel_spmd

mdt = mybir.dt
AT = mybir.ActivationFunctionType
ALU = mybir.AluOpType
AX = mybir.AxisListType

B, V, D = 8, 5000, 10
EMB, F, E, H = 32, 2, 64, 4
DH = E // H
VP, T = 5120, 40
LAYERS, FLOW_ITERS, DUAL_ITERS = 2, 10, 10
STEP, MOM = 0.01, 0.9
BIG = 1e9
CH = 4            # node tiles per chunk
NCHUNK = T // CH  # 10
ISQ = 1.0 / 4.0   # 1/sqrt(dh)
LS_CHAINS = 4     # local_scatter chain passes (measured max dup depth 3, +1 spare)


# ---------------- host-side layout helpers ----------------

def wrap16(lst):
    """dma_gather index layout: list[i] -> sbuf[i % 16, i // 16], replicated
    across the 8 gpsimd partition groups."""
    a = np.asarray(lst, np.int16)
    n = len(a)
    assert n % 16 == 0
    a = a.reshape(n // 16, 16).T  # [16, n/16]
    return np.ascontiguousarray(np.tile(a, (8, 1)))  # [128, n/16]


def groupwrap16(lists):
    """ap_gather per-core lists: lists[g] wrapped into partitions 16g..16g+15."""
    rows = []
    for g in range(8):
        a = np.asarray(lists[g], np.int16)
        rows.append(a.reshape(len(a) // 16, 16).T)
    return np.ascontiguousarray(np.concatenate(rows, axis=0))


def node_major(a2d):
    c = a2d.shape[1]
    out = np.zeros((VP, c), a2d.dtype)
    out[: a2d.shape[0]] = a2d
    return np.ascontiguousarray(
        out.reshape(T, 128, c).transpose(1, 0, 2).reshape(128, T * c)
    )


def mcf_major(a1d):
    out = np.zeros(VP, a1d.dtype)
    out[: a1d.shape[0]] = a1d
    return np.ascontiguousarray(out.reshape(128, 40))


def edge_list_chunks(src):
    """src [VP, D] int -> flat gather list with chunk-of-CH-tiles node-major order."""
    lst = np.empty(VP * D, np.int64)
    for c in range(NCHUNK):
        base = c * CH * 128 * D
        for tt in range(CH):
            t = c * CH + tt
            for nb in range(D):
                j = tt * D + nb
                lst[base + j * 128 : base + (j + 1) * 128] = src[t * 128 : (t + 1) * 128, nb]
    return lst


def _shared_prep(inputs):
    import ml_dtypes

    f32 = np.float32

    def b16(x):
        return np.ascontiguousarray(np.asarray(x, f32).astype(ml_dtypes.bfloat16))

    emb = np.asarray(inputs["node_embedding_var"], f32)
    s = {}
    embT = np.zeros((EMB, VP), f32)
    embT[:, :V] = emb.T
    s["embT"] = embT
    s["embN"] = node_major(emb)
    W_enc = np.asarray(inputs["W_enc"], f32)
    s["Wenc1"] = np.ascontiguousarray(W_enc[:EMB])
    s["Wenc2"] = np.ascontiguousarray(W_enc[EMB:])
    s["benc"] = np.asarray(inputs["b_enc"], f32)[None]
    s["Wqkv"] = b16(np.concatenate([inputs["Wq"], inputs["Wk"], inputs["Wv"]], 1))
    s["Wo16"] = b16(inputs["Wo"])
    s["Wgru"] = b16(inputs["W_gru"])
    s["Ugru"] = b16(inputs["U_gru"])
    s["bgru"] = np.asarray(inputs["b_gru"], f32)[None]
    Wd1 = np.asarray(inputs["Wd1"], f32)
    s["Wdec"] = b16(np.concatenate([Wd1[:E], Wd1[E:], np.asarray(inputs["Wu1"], f32)], 1))
    s["bdec"] = np.concatenate(
        [np.zeros(32, f32), np.zeros(32, f32), np.asarray(inputs["bu1"], f32)]
    )[None]  # u1: no bias; y: bd1 folded into yd rows; udual: bu1
    s["Wd2rep"] = np.ascontiguousarray(np.tile(np.asarray(inputs["Wd2"], f32).T, (128, 1)))
    s["Wu2rep"] = np.ascontiguousarray(np.tile(np.asarray(inputs["Wu2"], f32).T, (128, 1)))
    s["bd2"] = float(np.asarray(inputs["bd2"]).reshape(-1)[0])
    s["bu2"] = float(np.asarray(inputs["bu2"]).reshape(-1)[0])
    bd1v = np.asarray(inputs["bd1"], f32)
    pad = np.zeros((120, 64), f32)
    pad[:, 0:32] = bd1v[None, :]
    s["ydpad120"] = pad
    s["identb"] = b16(np.eye(128, dtype=f32))
    s["ones1"] = np.ones((1, 128), f32)
    return s


def _host_prep(core, shared):
    f32 = np.float32
    import ml_dtypes

    demands = np.asarray(core["demands"], f32)[:, 0]
    feat = np.asarray(core["node_features"], f32)
    adj = np.asarray(core["adj_lst"], np.int64)
    in_idx = np.asarray(core["in_indices"], np.int64)
    rev_idx = np.asarray(core["rev_indices"], np.int64)
    num_nodes = int(core["num_nodes"])

    m = dict(shared)
    m["demN"] = node_major(demands[:, None])
    m["demM"] = mcf_major(demands)
    featT = np.zeros((F, VP), f32)
    featT[:, :V] = feat.T
    m["featT"] = featT

    maskf = (adj == num_nodes).astype(f32)
    m["maskN"] = node_major(maskf)
    m["nbigN"] = node_major(maskf * BIG)
    pm = np.zeros((VP, 1), f32)
    pm[:V] = 1.0
    m["pmaskT"] = node_major(pm)
    m["pmaskN"] = node_major(np.repeat(pm, D, axis=1))

    adjc = np.full((VP, D), V, np.int64)
    adjc[:V] = adj
    m["adjIdx"] = wrap16(edge_list_chunks(adjc))

    iv = np.zeros((VP, D), np.int64)
    iv[:V] = in_idx[..., 1]
    ip = np.zeros((VP, D), np.int64)
    ip[:V] = in_idx[..., 2]
    riv = np.zeros((VP, D), np.int64)
    riv[:V] = rev_idx[..., 1]
    rip = np.zeros((VP, D), np.int64)
    rip[:V] = rev_idx[..., 2]

    # local_scatter indices for the s-iteration: partition p owns dest nodes
    # [40p, 40p+40); edge (v, d) lives at slot (v%40)*10 + d. Pass 1 scatters
    # table position u -> first slot wanting s[u]; chain pass k copies slot of
    # use k to slot of use k+1 within the same (p, u) cell.
    v_arr = np.repeat(np.arange(V), D)
    d_arr = np.tile(np.arange(D), V)
    u_arr = iv[:V].reshape(-1)
    p_arr = v_arr // 40
    slot_arr = (v_arr % 40) * 10 + d_arr
    key = p_arr * VP + u_arr
    order = np.argsort(key, kind="stable")
    ks, sl = key[order], slot_arr[order]
    new_grp = np.r_[True, ks[1:] != ks[:-1]]
    grp_start = np.flatnonzero(new_grp)
    grp_id = np.cumsum(new_grp) - 1
    rank = np.arange(len(ks)) - grp_start[grp_id]
    assert rank.max() <= LS_CHAINS, f"dup chain depth {rank.max()} > {LS_CHAINS}"
    ls1 = np.full((128, VP), -1, np.int16)
    lsC = np.full((LS_CHAINS, 128, 400), -1, np.int16)
    m0 = rank == 0
    ls1[ks[m0] // VP, ks[m0] % VP] = sl[m0]
    for k in range(1, int(rank.max()) + 1):
        mk = rank == k
        pos = np.flatnonzero(mk)
        lsC[k - 1, ks[mk] // VP, sl[pos - 1]] = sl[mk]
    m["lsIdx1"] = ls1
    m["lsIdxC"] = np.ascontiguousarray(lsC.transpose(1, 0, 2).reshape(128, LS_CHAINS * 400))

    # pair-gather indices for the edge-weight (wM, mcf-major) and reverse-flow
    # (rvf, node-major) gathers. Flat fp16 table order e = (v%128)*400 +
    # (v//128)*10 + d (contiguous per node-major partition); ap_gather fetches
    # fp16 pairs at e>>1, parity masks select the half.
    dd = np.arange(D)[None, None, :]
    # mcf-major: partition P owns nodes 40P+s
    vv = 40 * np.arange(128)[:, None, None] + np.arange(40)[None, :, None]  # [128,40,1]
    uw = iv[vv, dd]  # [128, 40, 10]
    ew = (uw % 128) * 400 + (uw // 128) * 10 + ip[vv, dd]
    wpair = (ew >> 1).astype(np.int16)
    m["wpIdx"] = groupwrap16(
        [wpair[16 * g : 16 * g + 16].transpose(1, 0, 2).reshape(6400) for g in range(8)]
    )
    wparO = (ew & 1).astype(f32).reshape(128, 400)
    m["wParO"] = wparO
    m["wParE"] = 1.0 - wparO
    # node-major: partition p owns nodes t*128+p
    vvn = np.arange(128)[:, None, None] + 128 * np.arange(40)[None, :, None]
    ur = riv[vvn, dd]
    er = (ur % 128) * 400 + (ur // 128) * 10 + rip[vvn, dd]
    rpair = (er >> 1).astype(np.int16)
    m["rpIdx"] = groupwrap16(
        [rpair[16 * g : 16 * g + 16].transpose(1, 0, 2).reshape(6400) for g in range(8)]
    )
    rparO = (er & 1).astype(f32).reshape(128, 400)
    m["rParO"] = rparO
    m["rParE"] = 1.0 - rparO
    return m


# ---------------- device program ----------------

def build(nc, bd2, bu2, phase=5):
    f32, bf = mdt.float32, mdt.bfloat16

    def din(name, shape, dt=f32):
        return nc.dram_tensor(name, list(shape), dt, kind="ExternalInput").ap()

    embT = din("embT", [EMB, VP])
    embN = din("embN", [128, T * EMB])
    featT = din("featT", [F, VP])
    demN = din("demN", [128, T])
    demM = din("demM", [128, T])
    maskN = din("maskN", [128, T * D])
    nbigN = din("nbigN", [128, T * D])
    pmaskT = din("pmaskT", [128, T])
    pmaskN = din("pmaskN", [128, T * D])
    adjIdx = din("adjIdx", [128, 3200], mdt.int16)
    lsIdx1 = din("lsIdx1", [128, VP], mdt.int16)
    lsIdxC = din("lsIdxC", [128, LS_CHAINS * 400], mdt.int16)
    wpIdx = din("wpIdx", [128, 400], mdt.int16)
    rpIdx = din("rpIdx", [128, 400], mdt.int16)
    wParE = din("wParE", [128, 400])
    wParO = din("wParO", [128, 400])
    rParE = din("rParE", [128, 400])
    rParO = din("rParO", [128, 400])
    Wenc1 = din("Wenc1", [EMB, E])
    Wenc2 = din("Wenc2", [F, E])
    benc = din("benc", [1, E])
    Wqkv = din("Wqkv", [E, 3 * E], bf)
    Wo16 = din("Wo16", [E, E], bf)
    Wgru = din("Wgru", [E, 3 * E], bf)
    Ugru = din("Ugru", [E, 3 * E], bf)
    bgru = din("bgru", [1, 3 * E])
    Wdec = din("Wdec", [E, 96], bf)
    bdec = din("bdec", [1, 96])
    Wd2rep = din("Wd2rep", [128, 32])
    Wu2rep = din("Wu2rep", [128, 32])
    ydpad120 = din("ydpad120", [120, 64])
    identB = din("identb", [128, 128], bf)
    ones1 = din("ones1", [1, 128])

    out = nc.dram_tensor("out", [1, 1], f32, kind="ExternalOutput").ap()
    dbg = nc.dram_tensor("dbg", [128, 2560], f32, kind="ExternalOutput").ap()

    with tile.TileContext(nc) as tc:
        import contextlib

        ctx = contextlib.ExitStack()
        sb = ctx.enter_context(tc.tile_pool(name="sb", bufs=1))
        dram = ctx.enter_context(tc.tile_pool(name="dram", bufs=1, space="DRAM"))
        midctx = contextlib.ExitStack()
        mid = midctx.enter_context(tc.tile_pool(name="mid", bufs=1))

        S, A, Vc, P, PE = nc.sync, nc.scalar, nc.vector, nc.gpsimd, nc.tensor

        # persistent state
        x = mid.tile([128, T * E], f32, name="x")
        u1 = mid.tile([128, T * 32], f32, name="u1")
        xT = mid.tile([E, VP], bf, name="xT")
        q = mid.tile([128, T * E], bf, name="q")
        maskf = sb.tile([128, T * D], f32, name="maskf")
        nbig = sb.tile([128, T * D], f32, name="nbig")
        pmT = sb.tile([128, T], f32, name="pmT")
        pmN = sb.tile([128, T * D], f32, name="pmN")
        zkv = mid.tile([128, 128], bf, name="zkv")
        onesb = sb.tile([1, 128], f32, name="onesb")
        identb = mid.tile([128, 128], bf, name="identb_sb")
        adjI = mid.tile([128, 3200], mdt.int16, name="adjI")
        nW = sb.tile([128, T * D], f32, name="nW")
        dvN = sb.tile([128, T], f32, name="dvN")
        ddf = sb.tile([128, T * D], f32, name="ddf")
        dp = sb.tile([128, T], f32, name="dp")
        dpM = sb.tile([128, T], f32, name="dpM")
        demNs = sb.tile([128, T], f32, name="demNs")
        wq16 = sb.tile([E, 3 * E], bf, name="wq16")
        wo16s = sb.tile([E, E], bf, name="wo16s")
        wg16 = sb.tile([E, 3 * E], bf, name="wg16")
        ug16 = sb.tile([E, 3 * E], bf, name="ug16")
        wd16 = sb.tile([E, 96], bf, name="wd16")
        bg = sb.tile([1, 3 * E], f32, name="bg")
        bdc = sb.tile([1, 96], f32, name="bdc")
        wd2r = sb.tile([128, 32], f32, name="wd2r")
        wu2r = sb.tile([128, 32], f32, name="wu2r")

        kvDs = [dram.tile([VP, 128], bf, name=f"kvD{l}") for l in range(LAYERS)]
        ydD = dram.tile([VP, E], f32, name="ydD")
        sfD = dram.tile([VP, 1], f32, name="sfD")
        sfD16 = dram.tile([VP, 1], mdt.float16, name="sfD16")
        wpD = dram.tile([VP * D, 1], mdt.float16, name="wpD")

        nidxreg = P.alloc_register("nidxreg")
        P.reg_mov(nidxreg, CH * D * 128)
        S.dma_start(onesb[:], ones1)
        S.dma_start(identb[:], identB)
        S.dma_start(adjI[:], adjIdx)
        S.dma_start(maskf[:], maskN)
        S.dma_start(nbig[:], nbigN)
        S.dma_start(pmT[:], pmaskT)
        S.dma_start(pmN[:], pmaskN)
        P.memset(zkv[:], 0.0)
        S.dma_start(wq16[:], Wqkv)
        S.dma_start(wo16s[:], Wo16)
        S.dma_start(wg16[:], Wgru)
        S.dma_start(ug16[:], Ugru)
        S.dma_start(bg[:], bgru)
        S.dma_start(wd16[:], Wdec)
        S.dma_start(bdc[:], bdec)
        S.dma_start(wd2r[:], Wd2rep)
        S.dma_start(wu2r[:], Wu2rep)
        S.dma_start(demNs[:], demN)

        # ---------------- encode ----------------
        with tc.tile_pool(name="encp", bufs=2) as enc, tc.tile_pool(
            name="encps", bufs=2, space="PSUM"
        ) as eps:
            embTs = enc.tile([EMB, VP], f32, name="embTs")
            featTs = enc.tile([F, VP], f32, name="featTs")
            embNs = enc.tile([128, T * EMB], f32, name="embNs")
            w1 = enc.tile([EMB, E], f32, name="w1")
            w2 = enc.tile([F, E], f32, name="w2")
            be = enc.tile([1, E], f32, name="be")
            demMs = enc.tile([128, T], f32, name="demMs")
            S.dma_start(embTs[:], embT)
            S.dma_start(featTs[:], featT)
            S.dma_start(embNs[:], embN)
            S.dma_start(w1[:], Wenc1)
            S.dma_start(w2[:], Wenc2)
            S.dma_start(be[:], benc)
            S.dma_start(demMs[:], demM)

            A.activation(dp[:], demNs[:], AT.Relu)
            A.activation(dpM[:], demMs[:], AT.Relu)

            sqv = enc.tile([128, T * EMB], f32, name="sqv")
            Vc.tensor_mul(sqv[:], embNs[:], embNs[:])
            n2 = enc.tile([128, T], f32, name="n2")
            Vc.reduce_sum(n2[:], sqv[:].rearrange("p (t c) -> p t c", c=EMB), axis=AX.X)
            nrm = enc.tile([128, T], f32, name="nrm")
            A.activation(nrm[:], n2[:], AT.Sqrt)
            Vc.tensor_scalar_max(out=nrm[:], in0=nrm[:], scalar1=1.0)
            scl = enc.tile([128, T], f32, name="scl")
            Vc.reciprocal(scl[:], nrm[:])

            for t in range(T):
                p1 = eps.tile([128, E], f32, name="p1")
                p2 = eps.tile([128, E], f32, name="p2")
                PE.matmul(p1[:], embTs[:, t * 128 : (t + 1) * 128], w1[:], start=True, stop=True)
                PE.matmul(p2[:], featTs[:, t * 128 : (t + 1) * 128], w2[:], start=True, stop=False)
                PE.matmul(p2[:], onesb[:], be[:], start=False, stop=True)
                A.activation(x[:, t * E : (t + 1) * E], p2[:], AT.Copy)
                Vc.scalar_tensor_tensor(
                    out=x[:, t * E : (t + 1) * E], in0=p1[:], scalar=scl[:, t : t + 1],
                    in1=x[:, t * E : (t + 1) * E], op0=ALU.mult, op1=ALU.add,
                )
                xb = enc.tile([128, E], bf, name="xb")
                A.activation(xb[:], x[:, t * E : (t + 1) * E], AT.Copy)
                xtp = eps.tile([E, 128], bf, name="xtp")
                PE.transpose(xtp[:], xb[:], identb[:])
                A.activation(xT[:, t * 128 : (t + 1) * 128], xtp[:], AT.Copy)

        if phase <= 1:
            S.dma_start(dbg[:], x[:])
            S.dma_start(out, x[0:1, 0:1])
            midctx.close()
            ctx.close()
            return nc
        # ---------------- graph layers ----------------
        layctx = contextlib.ExitStack()
        kvps = layctx.enter_context(tc.tile_pool(name="kvps", bufs=3, space="PSUM"))
        for layer in range(LAYERS):
            with tc.tile_pool(name=f"lay{layer}", bufs=4) as lp, tc.tile_pool(
                name=f"lps{layer}", bufs=1, space="PSUM"
            ) as lps:
                kvD = kvDs[layer]

                def kv_chunk(c, dstD):
                    kvc = lp.tile([128, CH, 128], bf, name="kvc")
                    for tt in range(CH):
                        t = c * CH + tt
                        pq = kvps.tile([128, 3 * E], f32, name="pq")
                        PE.matmul(pq[:], xT[:, t * 128 : (t + 1) * 128], wq16[:], start=True, stop=True)
                        A.activation(q[:, t * E : (t + 1) * E], pq[:, :E], AT.Copy)
                        Vc.tensor_copy(kvc[:, tt, :], pq[:, E:])
                    S.dma_start(
                        dstD[:].rearrange("(t p) c -> p t c", p=128)[:, c * CH : (c + 1) * CH, :],
                        kvc[:],
                    )

                for c in range(NCHUNK):
                    kv_chunk(c, kvD)
                S.dma_start(
                    kvD[:].rearrange("(t p) c -> p t c", p=128)[8:128, T - 1, :],
                    zkv[8:128, :],
                )

                for c in range(NCHUNK):
                    kvn = lp.tile([128, CH * D, 128], bf, name="kvn")
                    P.dma_gather(
                        kvn[:], kvD[:], adjI[:, c * 320 : (c + 1) * 320],
                        num_idxs=CH * D * 128, num_idxs_reg=nidxreg, elem_size=128, single_packet=False,
                    )
                    # scores
                    prodk = lp.tile([128, CH * D * E], bf, name="prodk")
                    qv = (
                        q[:, c * CH * E : (c + 1) * CH * E]
                        .rearrange("p (tt e) -> p tt e", e=E)
                        .unsqueeze(2)
                        .broadcast_to((128, CH, D, E))
                    )
                    Vc.tensor_mul(
                        prodk[:].rearrange("p (tt nb e) -> p tt nb e", nb=D, e=E),
                        kvn[:, :, 0:E].rearrange("p (tt nb) e -> p tt nb e", nb=D),
                        qv,
                    )
                    sc = lp.tile([128, CH * D * H], f32, name="sc")
                    Vc.reduce_sum(
                        sc[:].rearrange("p (tn h) -> p tn h", h=H),
                        prodk[:].rearrange("p (tnh dh) -> p tnh dh", dh=DH),
                        axis=AX.X,
                    )
                    scm = lp.tile([128, CH * D * H], f32, name="scm")
                    Vc.scalar_tensor_tensor(
                        out=scm[:].rearrange("p (tn h) -> p tn h", h=H),
                        in0=sc[:].rearrange("p (tn h) -> p tn h", h=H),
                        scalar=ISQ,
                        in1=nbig[:, c * CH * D : (c + 1) * CH * D].unsqueeze(2).broadcast_to((128, CH * D, H)),
                        op0=ALU.mult, op1=ALU.subtract,
                    )
                    ex = lp.tile([128, CH * D * H], f32, name="ex")
                    A.activation(ex[:], scm[:], AT.Exp)
                    zs = lp.tile([128, CH * H], f32, name="zs")
                    Vc.reduce_sum(
                        zs[:].rearrange("p (tt h) -> p tt h", h=H),
                        ex[:].rearrange("p (tt nb h) -> p tt h nb", nb=D, h=H),
                        axis=AX.X,
                    )
                    rz = lp.tile([128, CH * H], f32, name="rz")
                    Vc.reciprocal(rz[:], zs[:])
                    at = lp.tile([128, CH * D * H], f32, name="at")
                    Vc.tensor_mul(
                        at[:].rearrange("p (tt nb h) -> p tt nb h", nb=D, h=H),
                        ex[:].rearrange("p (tt nb h) -> p tt nb h", nb=D, h=H),
                        rz[:].rearrange("p (tt h) -> p tt h", h=H).unsqueeze(2).broadcast_to((128, CH, D, H)),
                    )
                    prodv = lp.tile([128, CH * D * E], f32, name="prodv")
                    Vc.tensor_mul(
                        prodv[:].rearrange("p (tt nb h dh) -> p tt nb h dh", nb=D, h=H, dh=DH),
                        kvn[:, :, E:].rearrange("p (tt nb) (h dh) -> p tt nb h dh", nb=D, h=H),
                        at[:].rearrange("p (tt nb h) -> p tt nb h", nb=D, h=H).unsqueeze(4).broadcast_to(
                            (128, CH, D, H, DH)
                        ),
                    )
                    agg = lp.tile([128, CH * E], f32, name="agg")
                    Vc.reduce_sum(
                        agg[:].rearrange("p (tt e) -> p tt e", e=E),
                        prodv[:].rearrange("p (tt nb e) -> p tt e nb", nb=D, e=E),
                        axis=AX.X,
                    )
                    # GRU per tile
                    for tt in range(CH):
                        t = c * CH + tt
                        aggb = lp.tile([128, E], bf, name="aggb")
                        A.activation(aggb[:], agg[:, tt * E : (tt + 1) * E], AT.Copy)
                        agT = lps.tile([E, 128], bf, name="trT")
                        PE.transpose(agT[:], aggb[:], identb[:])
                        agTs = lp.tile([E, 128], bf, name="agTs")
                        A.activation(agTs[:], agT[:], AT.Copy)
                        pnx = lps.tile([128, E], f32, name="pnx")
                        PE.matmul(pnx[:], agTs[:], wo16s[:], start=True, stop=True)
                        nxt = lp.tile([128, E], bf, name="nxt")
                        A.activation(nxt[:], pnx[:], AT.Tanh)
                        nxT = lps.tile([E, 128], bf, name="trT")
                        PE.transpose(nxT[:], nxt[:], identb[:])
                        nxTs = lp.tile([E, 128], bf, name="nxTs")
                        A.activation(nxTs[:], nxT[:], AT.Copy)
                        pA = lps.tile([128, 2 * E], f32, name="pA")
                        PE.matmul(pA[:], nxTs[:], wg16[:, : 2 * E], start=True, stop=False)
                        PE.matmul(pA[:], xT[:, t * 128 : (t + 1) * 128], ug16[:, : 2 * E], start=False, stop=False)
                        PE.matmul(pA[:], onesb[:], bg[:, : 2 * E], start=False, stop=True)
                        pBC = lps.tile([128, 2 * E], f32, name="pBC")
                        PE.matmul(pBC[:, :E], nxTs[:], wg16[:, 2 * E :], start=True, stop=False)
                        PE.matmul(pBC[:, :E], onesb[:], bg[:, 2 * E :], start=False, stop=True)
                        PE.matmul(pBC[:, E:], xT[:, t * 128 : (t + 1) * 128], ug16[:, 2 * E :], start=True, stop=True)
                        zr = lp.tile([128, 2 * E], f32, name="zr")
                        A.activation(zr[:], pA[:], AT.Sigmoid)
                        tmp = lp.tile([128, E], f32, name="tmp")
                        Vc.tensor_mul(tmp[:], zr[:, E:], pBC[:, E:])
                        Vc.tensor_add(out=tmp[:], in0=tmp[:], in1=pBC[:, :E])
                        hh = lp.tile([128, E], f32, name="hh")
                        A.activation(hh[:], tmp[:], AT.Tanh)
                        hmx = lp.tile([128, E], f32, name="hmx")
                        Vc.tensor_sub(out=hmx[:], in0=hh[:], in1=x[:, t * E : (t + 1) * E])
                        Vc.tensor_mul(hmx[:], hmx[:], zr[:, :E])
                        Vc.tensor_add(
                            out=x[:, t * E : (t + 1) * E], in0=x[:, t * E : (t + 1) * E], in1=hmx[:]
                        )
                        xb2 = lp.tile([128, E], bf, name="xb2")
                        A.activation(xb2[:], x[:, t * E : (t + 1) * E], AT.Copy)
                        xtp2 = lps.tile([E, 128], bf, name="xtp2")
                        PE.transpose(xtp2[:], xb2[:], identb[:])
                        A.activation(xT[:, t * 128 : (t + 1) * 128], xtp2[:], AT.Copy)

        layctx.close()
        if phase <= 2:
            S.dma_start(dbg[:], x[:])
            S.dma_start(out, x[0:1, 0:1])
            midctx.close()
            ctx.close()
            return nc
        # ---------------- decoders ----------------
        with tc.tile_pool(name="decp", bufs=3) as dpool, tc.tile_pool(
            name="decps", bufs=2, space="PSUM"
        ) as dps:
            for c in range(NCHUNK):
                ydc = dpool.tile([128, CH, 33], f32, name="ydc")
                for tt in range(CH):
                    t = c * CH + tt
                    pd = dps.tile([128, 96], f32, name="pd")
                    PE.matmul(pd[:], xT[:, t * 128 : (t + 1) * 128], wd16[:], start=True, stop=False)
                    PE.matmul(pd[:], onesb[:], bdc[:], start=False, stop=True)
                    Vc.tensor_copy(u1[:, t * 32 : (t + 1) * 32], pd[:, 0:32])
                    Vc.tensor_copy(ydc[:, tt, 0:32], pd[:, 32:64])
                    th = dpool.tile([128, 32], f32, name="th")
                    A.activation(th[:], pd[:, 64:96], AT.Tanh)
                    junk = dpool.tile([128, 32], f32, name="junk")
                    Vc.tensor_mul(junk[:], th[:], wu2r[:])
                    Vc.reduce_sum(dvN[:, t : t + 1], junk[:], axis=AX.X)
                    Vc.tensor_scalar_add(out=dvN[:, t : t + 1], in0=dvN[:, t : t + 1], scalar1=bu2)
                    Vc.tensor_copy(ydc[:, tt, 32:33], dvN[:, t : t + 1])
                S.dma_start(
                    ydD[:].rearrange("(t p) c -> p t c", p=128)[:, c * CH : (c + 1) * CH, 0:33],
                    ydc[:],
                )
            # pad rows 5000..5119 = [bd1 | 0]
            ydp = dpool.tile([120, 64], f32, name="ydp")
            S.dma_start(ydp[:], ydpad120)
            S.dma_start(ydD[:].rearrange("(t p) c -> p t c", p=128)[8:128, T - 1, :], ydp[:])

            for c in range(NCHUNK):
                ydg = dpool.tile([128, CH * D, E], f32, name="ydg")
                P.dma_gather(
                    ydg[:], ydD[:], adjI[:, c * 320 : (c + 1) * 320],
                    num_idxs=CH * D * 128, num_idxs_reg=nidxreg, elem_size=E, single_packet=False,
                )
                h1p = dpool.tile([128, CH * D * 32], f32, name="h1p")
                u1v = (
                    u1[:, c * CH * 32 : (c + 1) * CH * 32]
                    .rearrange("p (tt k) -> p tt k", k=32)
                    .unsqueeze(2)
                    .broadcast_to((128, CH, D, 32))
                )
                h1p4 = h1p[:].rearrange("p (tt nb k) -> p tt nb k", nb=D, k=32)
                mk4 = (
                    maskf[:, c * CH * D : (c + 1) * CH * D]
                    .rearrange("p (tt nb) -> p tt nb", nb=D)
                    .unsqueeze(3)
                    .broadcast_to((128, CH, D, 32))
                )
                # h1p = u1*maskf ; then u1 - u1*maskf ; then + ydg
                Vc.tensor_mul(h1p4, u1v, mk4)
                Vc.tensor_sub(out=h1p4, in0=u1v, in1=h1p4)
                Vc.tensor_add(
                    out=h1p4,
                    in0=h1p4,
                    in1=ydg[:, :, 0:32].rearrange("p (tt nb) k -> p tt nb k", nb=D),
                )
                h1t = dpool.tile([128, CH * D * 32], f32, name="h1t")
                A.activation(h1t[:], h1p[:], AT.Tanh)
                pw = dpool.tile([128, CH * D * 32], f32, name="pw")
                Vc.tensor_mul(
                    pw[:].rearrange("p (tn k) -> p tn k", k=32),
                    h1t[:].rearrange("p (tn k) -> p tn k", k=32),
                    wd2r[:].unsqueeze(1).broadcast_to((128, CH * D, 32)),
                )
                nwc = dpool.tile([128, CH * D], f32, name="nwc")
                Vc.reduce_sum(nwc[:], pw[:].rearrange("p (tn k) -> p tn k", k=32), axis=AX.X)
                prd = dpool.tile([128, CH * D], f32, name="prd")
                Vc.tensor_sub(out=prd[:], in0=nwc[:], in1=nbig[:, c * CH * D : (c + 1) * CH * D])
                exn = dpool.tile([128, CH * D], f32, name="exn")
                A.activation(exn[:], prd[:], AT.Exp)
                zn = dpool.tile([128, CH], f32, name="zn")
                Vc.reduce_sum(zn[:], exn[:].rearrange("p (tt nb) -> p tt nb", nb=D), axis=AX.X)
                Vc.tensor_scalar_add(out=zn[:], in0=zn[:], scalar1=1e-30)
                rzn = dpool.tile([128, CH], f32, name="rzn")
                Vc.reciprocal(rzn[:], zn[:])
                Vc.tensor_mul(
                    nW[:, c * CH * D : (c + 1) * CH * D].rearrange("p (tt nb) -> p tt nb", nb=D),
                    exn[:].rearrange("p (tt nb) -> p tt nb", nb=D),
                    rzn[:].unsqueeze(2).broadcast_to((128, CH, D)),
                )
                # dual_diff = ydg[:, :, 32] - maskf*dv
                Vc.tensor_mul(
                    ddf[:, c * CH * D : (c + 1) * CH * D].rearrange("p (tt nb) -> p tt nb", nb=D),
                    maskf[:, c * CH * D : (c + 1) * CH * D].rearrange("p (tt nb) -> p tt nb", nb=D),
                    dvN[:, c * CH : (c + 1) * CH].unsqueeze(2).broadcast_to((128, CH, D)),
                )
                Vc.tensor_sub(
                    out=ddf[:, c * CH * D : (c + 1) * CH * D],
                    in0=ydg[:, :, 32],
                    in1=ddf[:, c * CH * D : (c + 1) * CH * D],
                )

        if phase <= 3:
            S.dma_start(dbg[:, 0:400], nW[:])
            S.dma_start(dbg[:, 400:800], ddf[:])
            S.dma_start(out, nW[0:1, 0:1])
            midctx.close()
            ctx.close()
            return nc
        # ---------------- mcf flow ----------------
        midctx.close()
        with tc.tile_pool(name="mcfp", bufs=1) as mp, tc.tile_pool(
            name="mcfps", bufs=2, space="PSUM"
        ) as mps:
            fp16 = mdt.float16
            from concourse.bass import _add_dep_helper

            lsI1s = mp.tile([128, VP], mdt.int16, name="lsI1s")
            S.dma_start(lsI1s[:], lsIdx1)
            lsICs = mp.tile([128, LS_CHAINS * 400], mdt.int16, name="lsICs")
            S.dma_start(lsICs[:], lsIdxC)
            wpIdxS = mp.tile([128, 400], mdt.int16, name="wpIdxS")
            S.dma_start(wpIdxS[:], wpIdx)
            rpIdxS = mp.tile([128, 400], mdt.int16, name="rpIdxS")
            S.dma_start(rpIdxS[:], rpIdx)
            wmE = mp.tile([128, 400], f32, name="wmE")
            S.dma_start(wmE[:], wParE)
            wmO = mp.tile([128, 400], f32, name="wmO")
            S.dma_start(wmO[:], wParO)
            rmE = mp.tile([128, 400], f32, name="rmE")
            S.dma_start(rmE[:], rParE)
            rmO = mp.tile([128, 400], f32, name="rmO")
            S.dma_start(rmO[:], rParO)

            wTab = mp.tile([128, VP * D], fp16, name="wTab")
            ev16 = mp.tile([128, 400], fp16, name="ev16")
            gpair = mp.tile([128, 12800], fp16, name="gpair")
            psel = mp.tile([128, 800], fp16, name="psel")

            def pair_gather(vals_f32, idx_tile, mE, mO, outA, outB):
                """flat fp16 table[e]: outA = even-half*mE, outB = odd-half*mO;
                caller sums. vals_f32 [128, 400] node-major edge values."""
                A.activation(ev16[:], vals_f32, AT.Copy)
                S.dma_start(wpD[:].rearrange("(p c) o -> p (c o)", p=128), ev16[:])
                S.dma_start(
                    wTab[:],
                    wpD[:].rearrange("(a v) o -> a (v o)", a=1).broadcast_to((128, VP * D)),
                )
                apg = P.ap_gather(
                    gpair[:], wTab[:], idx_tile[:],
                    channels=128, num_elems=VP * D // 2, d=2, num_idxs=6400,
                )
                for r in range(16):
                    eng = S if r % 2 == 0 else A
                    dmi = eng.dma_start(
                        psel[r:128:16, :].rearrange("p (s d c) -> p s d c", d=D, c=2),
                        gpair[r:128:16, :].rearrange(
                            "p (s r2 d c) -> p s r2 d c", r2=16, d=D, c=2
                        )[:, :, r, :, :],
                    )
                    _add_dep_helper(dmi.ins, apg.ins, sync=True, reason="extract")
                pv = psel[:].rearrange("p (s c) -> p s c", c=2)
                Vc.tensor_mul(outA, pv[:, :, 0], mE[:])
                Vc.tensor_mul(outB, pv[:, :, 1], mO[:])

            wM = mp.tile([128, 400], f32, name="wM")
            wMo = mp.tile([128, 400], f32, name="wMo")
            pair_gather(nW[:], wpIdxS, wmE, wmO, wM[:], wMo[:])
            Vc.tensor_add(out=wM[:], in0=wM[:], in1=wMo[:])
            fp16 = mdt.float16
            sTab16 = mp.tile([128, VP], fp16, name="sTab16")
            sM16 = mp.tile([128, T], fp16, name="sM16")
            sM = mp.tile([128, T], f32, name="sM")

            def rebuild_table16(src_mcf):
                A.activation(sM16[:], src_mcf, AT.Copy)
                S.dma_start(sfD16[:].rearrange("(p s) o -> p (s o)", p=128), sM16[:])
                S.dma_start(
                    sTab16[:],
                    sfD16[:].rearrange("(a v) o -> a (v o)", a=1).broadcast_to((128, VP)),
                )

            Vc.tensor_copy(sM[:], dpM[:])
            rebuild_table16(sM[:])
            sels = [mp.tile([128, 400], fp16, name=f"sel{k}") for k in range(LS_CHAINS + 1)]
            accF = mp.tile([128, 400], f32, name="accF")
            tn = mp.tile([128, 400], f32, name="tn")
            tm = mp.tile([128, T], f32, name="tm")
            for it in range(FLOW_ITERS):
                P.local_scatter(
                    sels[0][:], sTab16[:], lsI1s[:], channels=128, num_elems=400, num_idxs=VP
                )
                for k in range(LS_CHAINS):
                    P.local_scatter(
                        sels[k + 1][:], sels[k][:], lsICs[:, k * 400 : (k + 1) * 400],
                        channels=128, num_elems=400, num_idxs=400,
                    )
                Vc.tensor_add(out=accF[:], in0=sels[0][:], in1=sels[1][:])
                for k in range(2, LS_CHAINS + 1):
                    Vc.tensor_add(out=accF[:], in0=accF[:], in1=sels[k][:])
                Vc.tensor_mul(tn[:], wM[:], accF[:])
                Vc.reduce_sum(tm[:], tn[:].rearrange("p (s d) -> p s d", d=D), axis=AX.X)
                Vc.tensor_add(out=sM[:], in0=tm[:], in1=dpM[:])
                if it < FLOW_ITERS - 1:
                    rebuild_table16(sM[:])

            # s mcf-major -> node-major
            sMv = mp.tile([128, T], f32, name="sMv")
            S.dma_start(sfD[:].rearrange("(p s) o -> p (s o)", p=128), sM[:])
            S.dma_start(sMv[:].unsqueeze(2), sfD[:].rearrange("(t p) o -> p t o", p=128))

            flow = sb.tile([128, T * D], f32, name="flow")
            Vc.tensor_mul(
                flow[:].rearrange("p (t d) -> p t d", d=D),
                nW[:].rearrange("p (t d) -> p t d", d=D),
                sMv[:].unsqueeze(2).broadcast_to((128, T, D)),
            )
            Vc.tensor_mul(flow[:], flow[:], pmN[:])

            # rev gather from flow values (node-major pair-gather)
            rvf = mp.tile([128, T * D], f32, name="rvf")
            rvfo = mp.tile([128, T * D], f32, name="rvfo")
            pair_gather(flow[:], rpIdxS, rmE, rmO, rvf[:], rvfo[:])
            Vc.tensor_add(out=rvf[:], in0=rvf[:], in1=rvfo[:])
            # relu(flow - min(flow, rvf)) == relu(flow - rvf) for flow, rvf >= 0
            Vc.tensor_sub(out=flow[:], in0=flow[:], in1=rvf[:])
            A.activation(flow[:], flow[:], AT.Relu)
            Vc.tensor_mul(flow[:], flow[:], pmN[:])

        if phase <= 4:
            S.dma_start(dbg[:, 0:400], flow[:])
            S.dma_start(out, flow[0:1, 0:1])
            ctx.close()
            return nc
        # ---------------- dual iters + costs ----------------
        with tc.tile_pool(name="dup", bufs=2) as up:
            dd01 = up.tile([128, T * D], f32, name="dd01")
            Vc.tensor_scalar_mul(out=dd01[:], in0=ddf[:], scalar1=STEP)
            fDu = up.tile([128, T * D], f32, name="fDu")
            acc = up.tile([128, T * D], f32, name="acc")
            P.memset(fDu[:], 0.0)
            P.memset(acc[:], 0.0)
            om = up.tile([128, T * D], f32, name="om")
            Vc.tensor_scalar(
                out=om[:], in0=maskf[:], scalar1=-1.0, scalar2=1.0, op0=ALU.mult, op1=ALU.add
            )
            for it in range(DUAL_ITERS):
                t2 = up.tile([128, T * D], f32, name="t2")
                Vc.scalar_tensor_tensor(
                    out=t2[:], in0=fDu[:], scalar=2.0 * STEP, in1=dd01[:], op0=ALU.mult, op1=ALU.add
                )
                Vc.scalar_tensor_tensor(
                    out=acc[:], in0=acc[:], scalar=MOM, in1=t2[:], op0=ALU.mult, op1=ALU.subtract
                )
                Vc.tensor_add(out=t2[:], in0=fDu[:], in1=acc[:])
                A.activation(t2[:], t2[:], AT.Relu)
                Vc.tensor_mul(fDu[:], t2[:], om[:])

            Vc.tensor_mul(fDu[:], fDu[:], pmN[:])
            Vc.tensor_mul(ddf[:], ddf[:], pmN[:])
            Vc.tensor_mul(dvN[:], dvN[:], pmT[:])
            junk2 = up.tile([128, T * D], f32, name="junk2")
            fc = up.tile([128, 1], f32, name="fc")
            Vc.tensor_mul(junk2[:], flow[:], flow[:])
            Vc.reduce_sum(fc[:], junk2[:], axis=AX.X)
            dtmp = up.tile([128, T * D], f32, name="dtmp")
            Vc.tensor_add(out=dtmp[:], in0=fDu[:], in1=ddf[:])
            dc = up.tile([128, 1], f32, name="dc")
            Vc.tensor_mul(junk2[:], fDu[:], dtmp[:])
            Vc.reduce_sum(dc[:], junk2[:], axis=AX.X)
            junk3 = up.tile([128, T], f32, name="junk3")
            ddem = up.tile([128, 1], f32, name="ddem")
            Vc.tensor_mul(junk3[:], dvN[:], demNs[:])
            Vc.reduce_sum(ddem[:], junk3[:], axis=AX.X)
            tot = up.tile([128, 1], f32, name="tot")
            Vc.tensor_sub(out=tot[:], in0=fc[:], in1=dc[:])
            Vc.tensor_add(out=tot[:], in0=tot[:], in1=ddem[:])
            totr = up.tile([128, 1], f32, name="totr")
            P.partition_all_reduce(totr[:], tot[:], channels=128, reduce_op=bass_isa.ReduceOp.add)
            S.dma_start(out, totr[0:1, :])
        ctx.close()
    return nc


_CACHE = {}


def _get_nc(bd2, bu2, phase=5):
    key = (round(bd2, 9), round(bu2, 9), phase)
    if key not in _CACHE:
        nc = bacc.Bacc("TRN2", target_bir_lowering=False, debug=False)
        build(nc, bd2, bu2, phase=phase)
        nc.compile()
        _CACHE[key] = nc
    return _CACHE[key]


def kernel(**inputs):
    shared = _shared_prep(inputs)
    bd2 = shared.pop("bd2")
    bu2 = shared.pop("bu2")
    nc = _get_nc(bd2, bu2)
    in_maps = []
    for b in range(B):
        core = {
            "demands": np.asarray(inputs["demands"][b]),
            "node_features": np.asarray(inputs["node_features"][b]),
            "adj_lst": np.asarray(inputs["adj_lst"][b]),
            "in_indices": np.asarray(inputs["in_indices"][b]),
            "rev_indices": np.asarray(inputs["rev_indices"][b]),
            "num_nodes": np.asarray(inputs["num_nodes"][b]),
        }
        in_maps.append(_host_prep(core, shared))
    res = run_bass_kernel_spmd(nc, in_maps, core_ids=list(range(B)))
    return np.array([res.results[b]["out"][0, 0] for b in range(B)], np.float32)


if __name__ == "__main__":
    import reference

    inputs = {k: np.asarray(v) for k, v in reference.setup_inputs().items()}
    expected = np.asarray(reference.reference(**{k: v for k, v in inputs.items()}))
    got = kernel(**inputs)
    print("expected:", expected)
    print("got:     ", got)
    err = np.abs(got - expected) / (np.abs(expected) + 1e-9)
    print("rel err: ", err.max())

